# revision 38
# baseline (speedup 1.0000x reference)
"""Trainium2 Bass kernel for nn_NodeClassifier (gnn_message_passing).

Strategy (8 NeuronCores, SPMD):
  - Nodes block-partitioned by id across 8 cores (6250 each, padded to 6272).
    Within each core's block, nodes are sorted by in-degree so that the
    padded neighbor grid (K-grid) is tight.
  - Edges partitioned by dst core. Per dst 128-node tile, neighbor src
    embeddings arrive FEATURE-MAJOR via one dma_gather(transpose=True) per
    tile from a replicated fp16 node table in DRAM (int16 indices biased by
    -32768 against a base shifted +32768 rows — the gather ucode does signed
    index math, so this addresses all 50176 rows). One 3D tensor_reduce per
    tile then yields the aggregated [feature x node] block directly.
  - Dense per-node compute (GCN linear, BN, FF) runs feature-major in bf16.
    BN affine transforms are folded into the adjacent matmuls (scaled W1 /
    diag residual / scaled cls weights), so only stats cross the BN
    boundary. BN statistics are AllReduced (tiny); the layer-0 output table
    is AllGathered in fp16 for layer 1's gathers. Weights replicated.
"""

import os
import sys
import numpy as np

for _p in ("/opt/trn_rl_repo",):
    if _p not in sys.path and os.path.isdir(_p):
        sys.path.insert(0, _p)

from contextlib import ExitStack

import ml_dtypes

import concourse.bass as bass
import concourse.bacc as bacc
import concourse.mybir as mybir
import concourse.tile as tile
from concourse.bass_utils import run_bass_kernel_spmd
from concourse.masks import make_identity

F32 = mybir.dt.float32
F16 = mybir.dt.float16
BF16 = mybir.dt.bfloat16
I16 = mybir.dt.int16
AF = mybir.ActivationFunctionType
ALU = mybir.AluOpType

NP_BF16 = ml_dtypes.bfloat16

CORES = 8
D = 128
H = 512
DEPTH = 2
EPS = 1e-5
CHUNK = 512  # node-chunk width for the dense phase (one PSUM bank fp32)
IBASE = 32768  # signed-int16 index bias for dma_gather


# ----------------------------------------------------------------------------
# Host-side preparation
# ----------------------------------------------------------------------------

def _prepare(nodes, edge_src, edge_dst):
    """Compute the permutation, sharding and gather schedules from edge data."""
    N = nodes.shape[0]
    assert N % CORES == 0
    sh_real = N // CORES
    nt = -(-sh_real // 128)
    sh = nt * 128
    if sh == sh_real:  # force at least one dummy slot (PAD token row must be 0)
        nt += 1
        sh += 128
    tok_n = CORES * sh
    assert tok_n <= 2 * IBASE, "token space must fit signed-int16 biased range"

    # a dummy (zero-row) token in the upper half of the table: its biased
    # int16 index is positive, so it is safe in the gather's trailing slots
    hi0_core = (IBASE + sh - 1) // sh  # first core whose block is >= IBASE
    pad_tok = hi0_core * sh + sh_real
    assert pad_tok >= IBASE

    deg = np.bincount(edge_dst, minlength=N).astype(np.int64)

    # permutation: per core block, sort nodes by degree ascending
    tok_of_node = np.empty(N, np.int64)
    node_of_tok = np.full(tok_n, -1, np.int64)
    for c in range(CORES):
        ids = np.arange(c * sh_real, (c + 1) * sh_real)
        order = np.argsort(deg[ids], kind="stable")
        toks = c * sh + np.arange(sh_real)
        tok_of_node[ids[order]] = toks
        node_of_tok[toks] = ids[order]

    # group edges by dst token
    dst_tok = tok_of_node[edge_dst]
    src_tok = tok_of_node[edge_src]
    order = np.argsort(dst_tok, kind="stable")
    dst_tok_s = dst_tok[order]
    src_tok_s = src_tok[order]
    cnt_tok = np.bincount(dst_tok_s, minlength=tok_n)
    start_tok = np.concatenate([[0], np.cumsum(cnt_tok)[:-1]])

    # shared K schedule: per tile index t, max over cores of max degree, even
    cnt_mat = cnt_tok.reshape(CORES, nt, 128)
    K_t = cnt_mat.max(axis=(0, 2))
    K_t = np.maximum(K_t, 2)
    K_t = K_t + (K_t % 2)
    koff = np.concatenate([[0], np.cumsum(K_t)])
    ksum = int(koff[-1])

    # per-core gather index grids [128, ksum] (partition = node slot%128)
    gidx = np.full((CORES, 128, ksum), pad_tok, np.int64)
    e_slot = dst_tok_s % sh  # slot within core
    e_core = dst_tok_s // sh
    e_t = e_slot // 128
    e_p = e_slot % 128
    e_r = np.arange(len(dst_tok_s)) - start_tok[dst_tok_s]  # rank within node
    e_col = koff[e_t] + e_r
    gidx[e_core, e_p, e_col] = src_tok_s

    # int16 biased gather index stream for layer 1, as k-major column groups:
    # a dma_gather(transpose=False) with index order i = k*128 + p fills the
    # node-major K-grid [128, cols, D] directly. A single-packet gather hangs
    # at >=1024 indices, so each tile's K columns are split into groups of at
    # most 6 real columns plus one trailing hi-pad column (the gather ucode
    # drops trailing negative indices, and pad rows are zero so the tree
    # reduce can include them). One extra pad column keeps each tile's total
    # column count even for the pairwise tree reduce.
    # Flat index position i lives at partition i%16, column i//16, replicated
    # across the 8 gpsimd q7 cores (partitions 16q+p).
    # The gather ucode drops trailing negative (= lower-half-token) indices,
    # so each instruction's final index must be positive. Where possible we
    # swap an upper-half token into node 127's last slot of the group (sums
    # are order-invariant per node); otherwise the group gets a hi-pad
    # column. Pad rows are zero so the tree reduce just includes them.
    KG = 7  # max columns per gather instruction (128*7 = 896 < 1024 cap)
    parts = []
    groups = []  # per tile: list of (k0, ncols_real, ncols_total, ioff16)
    Kp_t = []  # per tile: total reduced columns (real + pads)
    off = 0
    for t in range(nt):
        K = int(K_t[t])
        kgs = []
        pads = []
        k0 = 0
        while k0 < K:
            kg = min(KG, K - k0)
            # can every core end this group on a positive index (via swap)?
            sl = gidx[:, 127, koff[t] + k0:koff[t] + k0 + kg]
            if (sl >= IBASE).any(axis=1).all():
                for c in range(CORES):
                    j = int(np.argmax(sl[c] >= IBASE))
                    last = kg - 1
                    sl[c, j], sl[c, last] = sl[c, last], sl[c, j]
                pad = 0
            else:
                if kg == KG:
                    kg -= 1
                pad = 1
            kgs.append(kg)
            pads.append(pad)
            k0 += kg
        if (K + sum(pads)) % 2:  # keep the tile's column count even
            for i in range(len(kgs)):
                if kgs[i] + pads[i] < KG:
                    pads[i] += 1
                    break
            else:
                kgs.append(0)
                pads.append(1)
        gl = []
        k0 = 0
        for kg, pad in zip(kgs, pads):
            blk = gidx[:, :, koff[t] + k0:koff[t] + k0 + kg]  # [C, 128, kg]
            blk = blk.transpose(0, 2, 1).reshape(CORES, 128 * kg)
            if pad:
                blk = np.concatenate(
                    [blk, np.full((CORES, 128 * pad), pad_tok, np.int64)],
                    axis=1)
            parts.append(blk)
            gl.append((k0, kg, kg + pad, off))
            off += 128 * (kg + pad) // 16
            k0 += kg
        groups.append(gl)
        Kp_t.append(sum(g[2] for g in gl))
    flat = np.concatenate(parts, axis=1)
    flat16 = (flat - IBASE).astype(np.int16)
    ncol16 = flat.shape[1] // 16
    gidx16 = np.zeros((CORES, 16, ncol16), np.int16)
    pos = np.arange(flat.shape[1])
    gidx16[:, pos % 16, pos // 16] = flat16
    gidx16 = np.tile(gidx16, (1, 8, 1))  # replicate for the 8 gpsimd cores

    # per-core invdeg [128, nt] fp32, node-slot partition layout (0 = dummy)
    deg_tok = cnt_tok.reshape(CORES, sh)
    node_ok = (node_of_tok.reshape(CORES, sh) >= 0)
    iv = (1.0 / np.maximum(deg_tok, 1.0)) * node_ok  # [CORES, sh]
    invdeg = np.zeros((CORES, 128, nt), np.float32)
    for c in range(CORES):
        invdeg[c] = iv[c].reshape(nt, 128).T

    # replicated full node table [tok_n, D], zero at dummy slots
    table0 = np.zeros((tok_n, D), np.float32)
    real = node_of_tok >= 0
    table0[real] = nodes[node_of_tok[real]]

    # host-expanded layer-0 gather payload, node-major [128, ksum*D] fp16
    t16 = table0.astype(np.float16)
    pay1 = t16[gidx]  # [CORES, 128, ksum, D]
    pay1 = np.ascontiguousarray(pay1.reshape(CORES, 128, ksum * D))

    return dict(
        N=N, sh_real=sh_real, sh=sh, nt=nt, tok_n=tok_n,
        K_t=[int(k) for k in K_t], koff=[int(k) for k in koff], ksum=ksum,
        Kp_t=Kp_t, groups=groups, gidx16=gidx16, invdeg=invdeg,
        table0=table0, pay1=pay1, node_of_tok=node_of_tok,
    )


# ----------------------------------------------------------------------------
# Program builder
# ----------------------------------------------------------------------------

def _emit_tree_reduce(nc, G16, G2, K, acc):
    """acc = sum of K [128,D] fp16 chunks of G16, all-fp16 pairwise tree.
    Pass 1 pairs halves of G16 into G2, then in-place halving on G2."""
    ALU_ = mybir.AluOpType
    half = K // 2  # K is even
    if half == 1:
        nc.vector.tensor_tensor(out=acc[:], in0=G16[:, :D],
                                in1=G16[:, D:2 * D], op=ALU_.add)
        return
    nc.vector.tensor_tensor(out=G2[:, :half * D], in0=G16[:, :half * D],
                            in1=G16[:, half * D:K * D], op=ALU_.add)
    width = half
    while width > 2:
        h = width // 2
        if width % 2:
            nc.vector.tensor_tensor(
                out=G2[:, 0:D], in0=G2[:, 0:D],
                in1=G2[:, (width - 1) * D:width * D], op=ALU_.add)
        if h == 1:  # width was 3: after the fold only chunks 0,1 remain
            break
        nc.vector.tensor_tensor(
            out=G2[:, :h * D], in0=G2[:, :h * D],
            in1=G2[:, h * D:2 * h * D], op=ALU_.add)
        width = h
    nc.vector.tensor_tensor(out=acc[:], in0=G2[:, 0:D], in1=G2[:, D:2 * D],
                            op=ALU_.add)


def build_program(cfg):
    nt, sh, sh_real = cfg["nt"], cfg["sh"], cfg["sh_real"]
    tok_n, ksum = cfg["tok_n"], cfg["ksum"]
    K_t, koff, groups = cfg["K_t"], cfg["koff"], cfg["groups"]
    Kp_t = cfg["Kp_t"]
    N = cfg["N"]
    ncol16 = cfg["gidx16"].shape[2]
    kmax = max(max(K_t), max(Kp_t))
    rg = [list(range(CORES))]

    chunks = []
    c0 = 0
    while c0 < sh:
        cw = min(CHUNK, sh - c0)
        chunks.append((c0, cw))
        c0 += cw
    nch = len(chunks)

    nc = bacc.Bacc("TRN2", target_bir_lowering=False, debug=False,
                   num_devices=CORES, num_swdge_queues=4)

    # ---- I/O declarations
    pay_d = nc.dram_tensor("pay1", [128, ksum * D], F16, kind="ExternalInput")
    x0_d = nc.dram_tensor("x0_fm", [D, sh], BF16, kind="ExternalInput")
    gidx_d = nc.dram_tensor("gidx16", [128, ncol16], I16, kind="ExternalInput")
    invdeg_d = nc.dram_tensor("invdeg", [128, nt], F32, kind="ExternalInput")
    wg_d = [nc.dram_tensor(f"wg{l}", [D, D], BF16, kind="ExternalInput")
            for l in range(DEPTH)]
    bgT_d = [nc.dram_tensor(f"bgT{l}", [1, D], BF16, kind="ExternalInput")
             for l in range(DEPTH)]
    w1_d = [nc.dram_tensor(f"w1_{l}", [D, H], BF16, kind="ExternalInput")
            for l in range(DEPTH)]
    fb1_d = [nc.dram_tensor(f"fb1_{l}", [D, H // D], F32, kind="ExternalInput")
             for l in range(DEPTH)]
    w2_d = [nc.dram_tensor(f"w2_{l}", [H, D], BF16, kind="ExternalInput")
            for l in range(DEPTH)]
    bn_d = {}
    for l in range(DEPTH):
        for nm in ("g1", "b1", "g2", "b2"):
            bn_d[(nm, l)] = nc.dram_tensor(f"{nm}_{l}", [D, 1], F32,
                                           kind="ExternalInput")
    clsw_d = nc.dram_tensor("clsw", [D, 16], BF16, kind="ExternalInput")
    clsb_d = nc.dram_tensor("clsb", [16, 1], F32, kind="ExternalInput")
    out_d = nc.dram_tensor("out_fm", [16, sh], F32, kind="ExternalOutput")

    with tile.TileContext(nc) as tc, ExitStack() as ctx:
        dram = ctx.enter_context(tc.tile_pool(name="dram", bufs=1, space="DRAM"))
        wp = ctx.enter_context(tc.tile_pool(name="weights", bufs=1))
        big = ctx.enter_context(tc.tile_pool(name="big", bufs=1))
        gp = ctx.enter_context(tc.tile_pool(name="gather", bufs=6))
        g2p = ctx.enter_context(tc.tile_pool(name="gred", bufs=2))
        pp = ctx.enter_context(tc.tile_pool(name="prep", bufs=2))
        sp = ctx.enter_context(tc.tile_pool(name="small", bufs=4))
        ck = ctx.enter_context(tc.tile_pool(name="chunk", bufs=2))
        psA = ctx.enter_context(tc.tile_pool(name="psA", bufs=1, space="PSUM"))
        psB = ctx.enter_context(tc.tile_pool(name="psB", bufs=2, space="PSUM"))

        # ---- internal DRAM (collective bounce buffers)
        vshard = dram.tile([sh, D], F16, name="vshard")
        vtab = dram.tile([tok_n, D], F16, addr_space="Shared", name="vtab")
        bn_in, bn_out = {}, {}
        for l in range(DEPTH):
            for j in (1, 2):
                bn_in[(l, j)] = dram.tile([D, 2], F32, name=f"bni{l}{j}")
                bn_out[(l, j)] = dram.tile([D, 2], F32, addr_space="Shared",
                                           name=f"bno{l}{j}")

        # ---- load constants / weights to SBUF
        def load(dt_, shape, src, name):
            t = wp.tile(shape, dt_, name=name)
            nc.sync.dma_start(out=t[:], in_=src)
            return t

        gidx_sb = load(I16, [128, ncol16], gidx_d[:], "gidx_sb")
        invdeg_sb = load(F32, [128, nt], invdeg_d[:], "invdeg_sb")
        wg_sb = [load(BF16, [D, D], wg_d[l][:], f"wg_sb{l}") for l in range(DEPTH)]
        bgT_sb = [load(BF16, [1, D], bgT_d[l][:], f"bgT_sb{l}") for l in range(DEPTH)]
        w1_sb = [load(BF16, [D, H], w1_d[l][:], f"w1_sb{l}") for l in range(DEPTH)]
        fb1_sb = [load(F32, [D, H // D], fb1_d[l][:], f"fb1_sb{l}")
                  for l in range(DEPTH)]
        w2_sb = [[load(BF16, [D, D], w2_d[l][h * D:(h + 1) * D, :], f"w2_sb{l}_{h}")
                  for h in range(H // D)] for l in range(DEPTH)]
        bn_sb = {k: load(F32, [D, 1], v[:], f"bn_{k[0]}_{k[1]}")
                 for k, v in bn_d.items()}
        clsw_sb = load(BF16, [D, 16], clsw_d[:], "clsw_sb")
        clsb_sb = load(F32, [16, 1], clsb_d[:], "clsb_sb")

        ident_b = wp.tile([128, 128], BF16, name="ident_b")
        make_identity(nc, ident_b[:])
        ones_row = wp.tile([1, CHUNK], BF16, name="ones_row")
        nc.vector.memset(ones_row[:], 1.0)
        qctr = [0]  # round-robin swdge queue assignment for gathers

        # ---- persistent full-width activations (feature-major [D, sh], bf16)
        bufA = big.tile([D, sh], BF16, name="bufA")  # agg both layers
        bufB = big.tile([D, sh], BF16, name="bufB")  # u(l0) -> xnew(l0)=xres(l1)
        bufC = big.tile([D, sh], BF16, name="bufC")  # x0(l0) -> v(l1)
        bufD = big.tile([D, sh], BF16, name="bufD")  # v(l0) -> u(l1)
        nc.sync.dma_start(out=bufC[:], in_=x0_d[:])

        def bn_vec_math(sums_sb, g_sb, b_sb, a_out, c_out):
            """a = g*rsqrt(var+eps); c = b - mean*a, from [D,2] (sum, sumsq)."""
            m = sp.tile([D, 1], F32, tag="bnv", name="m")
            msq = sp.tile([D, 1], F32, tag="bnv", name="msq")
            var = sp.tile([D, 1], F32, tag="bnv", name="var")
            r = sp.tile([D, 1], F32, tag="bnv", name="r")
            nc.vector.tensor_scalar_mul(out=m[:], in0=sums_sb[:, 0:1],
                                        scalar1=1.0 / N)
            nc.vector.tensor_scalar_mul(out=msq[:], in0=sums_sb[:, 1:2],
                                        scalar1=1.0 / N)
            nc.vector.tensor_tensor(out=var[:], in0=m[:], in1=m[:], op=ALU.mult)
            nc.vector.tensor_tensor(out=var[:], in0=msq[:], in1=var[:],
                                    op=ALU.subtract)
            nc.vector.tensor_scalar_add(out=var[:], in0=var[:], scalar1=EPS)
            nc.vector.reciprocal(out=r[:], in_=var[:])
            nc.scalar.activation(out=a_out[:], in_=r[:], func=AF.Sqrt)
            nc.vector.tensor_tensor(out=a_out[:], in0=g_sb[:], in1=a_out[:],
                                    op=ALU.mult)
            nc.vector.tensor_tensor(out=c_out[:], in0=m[:], in1=a_out[:],
                                    op=ALU.mult)
            nc.vector.tensor_tensor(out=c_out[:], in0=b_sb[:], in1=c_out[:],
                                    op=ALU.subtract)

        def finish_stats(ssum, ssq, l, j, a_out, c_out):
            """Reduce per-chunk partials, AllReduce, compute affine coefs."""
            s2 = sp.tile([D, 2], F32, tag="s2", name=f"s2_{l}{j}")
            nc.vector.tensor_reduce(out=s2[:, 0:1], in_=ssum[:],
                                    axis=mybir.AxisListType.X, op=ALU.add)
            nc.vector.tensor_reduce(out=s2[:, 1:2], in_=ssq[:],
                                    axis=mybir.AxisListType.X, op=ALU.add)
            nc.sync.dma_start(out=bn_in[(l, j)][:], in_=s2[:])
            nc.gpsimd.collective_compute(
                "AllReduce", ALU.add, replica_groups=rg,
                ins=[bn_in[(l, j)][:]], outs=[bn_out[(l, j)][:]])
            sums = sp.tile([D, 2], F32, tag="s2", name=f"sums{l}{j}")
            nc.sync.dma_start(out=sums[:], in_=bn_out[(l, j)][:])
            bn_vec_math(sums, bn_sb[(f"g{j}", l)], bn_sb[(f"b{j}", l)],
                        a_out, c_out)

        for l in range(DEPTH):
            agg = bufA
            if l == 0:
                u, xres, v = bufB, bufC, bufD
            else:
                u, xres, v = bufD, bufB, bufC

            # ---- dense sweep 1 chunk emitter (interleaved with aggregation
            # so the PE/Scalar in-order queues overlap the gather phase)
            ssum1 = sp.tile([D, nch], F32, tag="stat", name=f"ssum{l}1")
            ssq1 = sp.tile([D, nch], F32, tag="stat", name=f"ssq{l}1")

            def emit_sweep1_chunk(ci, l=l, u=u, xres=xres, agg=agg,
                                  ssum1=ssum1, ssq1=ssq1):
                c0, cw = chunks[ci]
                sl = slice(c0, c0 + cw)
                rw = max(0, min(cw, sh_real - c0))
                ph = psA.tile([D, CHUNK], F32, tag="mm_gcn", name=f"ph{l}{c0}")
                nc.tensor.matmul(ph[:, :cw], wg_sb[l][:], agg[:, sl],
                                 start=True, stop=False)
                nc.tensor.matmul(ph[:, :cw], bgT_sb[l][:],
                                 ones_row[:, :cw], start=False, stop=False)
                nc.tensor.matmul(ph[:, :cw], ident_b[:], xres[:, sl],
                                 start=False, stop=True)
                if rw == 0:
                    nc.vector.memset(ssum1[:, ci:ci + 1], 0.0)
                    nc.vector.memset(ssq1[:, ci:ci + 1], 0.0)
                    nc.scalar.activation(out=u[:, sl], in_=ph[:, :cw],
                                         func=AF.Copy)
                    return
                nc.scalar.activation(out=u[:, c0:c0 + rw], in_=ph[:, :rw],
                                     func=AF.Copy,
                                     accum_out=ssum1[:, ci:ci + 1])
                if rw < cw:
                    nc.scalar.activation(out=u[:, c0 + rw:c0 + cw],
                                         in_=ph[:, rw:cw], func=AF.Copy)
                sq = ck.tile([D, CHUNK], BF16, tag="sq", name=f"sq{l}1{ci}")
                nc.scalar.activation(out=sq[:, :rw], in_=u[:, c0:c0 + rw],
                                     func=AF.Square,
                                     accum_out=ssq1[:, ci:ci + 1])

            # ---- aggregation: node-major K-grid gather + fp16 tree reduce
            # + invdeg scale + PE transpose to feature-major
            next_chunk = 0
            for t in range(nt):
                K = K_t[t] if l == 0 else Kp_t[t]
                G16 = gp.tile([128, kmax * D], F16, tag="G16", name=f"G{l}_{t}")
                if l == 0:
                    nc.sync.dma_start(
                        out=G16[:, :K * D],
                        in_=pay_d[:, koff[t] * D:(koff[t] + K) * D])
                else:
                    goff = 0
                    for (k0, kg, ncol, io) in groups[t]:
                        nc.gpsimd.dma_gather(
                            out_ap=G16[:, goff * D:(goff + ncol) * D].rearrange(
                                "p (c d) -> p c d", c=ncol, d=D),
                            in_ap=vtab[IBASE:, :],
                            idxs_ap=gidx_sb[:, io:io + 128 * ncol // 16],
                            num_idxs=128 * ncol,
                            num_idxs_reg=128 * ncol,
                            elem_size=D,
                            transpose=False,
                            queue_num=qctr[0] % 4,
                        )
                        qctr[0] += 1
                        goff += ncol
                G2 = g2p.tile([128, (kmax // 2) * D], F16, tag="G2",
                              name=f"G2_{l}_{t}")
                acc = sp.tile([128, D], F16, tag="acc", name=f"acc{l}_{t}")
                with nc.allow_low_precision(reason="fp16 neighbor sums"):
                    _emit_tree_reduce(nc, G16, G2, K, acc)
                acc2 = sp.tile([128, D], BF16, tag="acc2", name=f"acc2{l}_{t}")
                nc.scalar.activation(out=acc2[:], in_=acc[:], func=AF.Copy,
                                     scale=invdeg_sb[:, t:t + 1])
                ps = psB.tile([128, 128], BF16, tag="tr", name=f"tr{l}_{t}")
                nc.tensor.transpose(ps[:], acc2[:], ident_b[:])
                nc.scalar.activation(out=agg[:, t * 128:(t + 1) * 128],
                                     in_=ps[:], func=AF.Copy)
                while (next_chunk < nch and
                       (chunks[next_chunk][0] + chunks[next_chunk][1] - 1)
                       // 128 <= t):
                    emit_sweep1_chunk(next_chunk)
                    next_chunk += 1

            a1 = sp.tile([D, 1], F32, tag="co", name=f"a1_{l}")
            c1 = sp.tile([D, 1], F32, tag="co", name=f"c1_{l}")
            finish_stats(ssum1, ssq1, l, 1, a1, c1)

            # ---- fold BN1 affine (x' = a1*u + c1) into the FF weights
            w1p = pp.tile([D, H], BF16, tag="w1p", name=f"w1p{l}")
            nc.vector.tensor_scalar_mul(out=w1p[:], in0=w1_sb[l][:],
                                        scalar1=a1[:])
            diag1 = pp.tile([128, 128], BF16, tag="diag", name=f"diag{l}")
            nc.vector.tensor_scalar_mul(out=diag1[:], in0=ident_b[:],
                                        scalar1=a1[:])
            c1b = pp.tile([D, 1], BF16, tag="cb", name=f"c1b{l}")
            nc.scalar.activation(out=c1b[:], in_=c1[:], func=AF.Copy)
            pfb = psA.tile([D, CHUNK], F32, tag="mm_ff1_0", name=f"pfb{l}")
            for h in range(H // D):
                nc.tensor.matmul(pfb[:, h:h + 1],
                                 w1_sb[l][:, h * D:(h + 1) * D], c1b[:],
                                 start=True, stop=True)
            fbp = pp.tile([D, H // D], F32, tag="fbp", name=f"fbp{l}")
            nc.vector.tensor_tensor(out=fbp[:], in0=pfb[:, :H // D],
                                    in1=fb1_sb[l][:], op=ALU.add)

            # ---- dense sweep 2: FF -> v = FF(x') + x' (+stats)
            ssum2 = sp.tile([D, nch], F32, tag="stat", name=f"ssum{l}2")
            ssq2 = sp.tile([D, nch], F32, tag="stat", name=f"ssq{l}2")
            for ci, (c0, cw) in enumerate(chunks):
                sl = slice(c0, c0 + cw)
                rw = max(0, min(cw, sh_real - c0))
                py = psA.tile([D, CHUNK], F32, tag="mm_ff2", name=f"py{l}{c0}")
                for h in range(H // D):
                    pr = psA.tile([D, CHUNK], F32, tag=f"mm_ff1_{h}",
                                  name=f"pr{l}{c0}{h}")
                    nc.tensor.matmul(pr[:, :cw], w1p[:, h * D:(h + 1) * D],
                                     u[:, sl], start=True, stop=True)
                    rh = ck.tile([D, CHUNK], BF16, tag=f"relu{h}",
                                 name=f"rh{l}{c0}{h}")
                    nc.scalar.activation(out=rh[:, :cw], in_=pr[:, :cw],
                                         func=AF.Relu, bias=fbp[:, h:h + 1],
                                         scale=1.0)
                    nc.tensor.matmul(py[:, :cw], w2_sb[l][h][:],
                                     rh[:, :cw], start=(h == 0), stop=False)
                nc.tensor.matmul(py[:, :cw], diag1[:], u[:, sl],
                                 start=False, stop=True)
                if rw == 0:
                    nc.vector.memset(ssum2[:, ci:ci + 1], 0.0)
                    nc.vector.memset(ssq2[:, ci:ci + 1], 0.0)
                    nc.scalar.activation(out=v[:, sl], in_=py[:, :cw],
                                         func=AF.Identity, bias=c1[:],
                                         scale=1.0)
                    continue
                nc.scalar.activation(out=v[:, c0:c0 + rw], in_=py[:, :rw],
                                     func=AF.Identity, bias=c1[:], scale=1.0,
                                     accum_out=ssum2[:, ci:ci + 1])
                if rw < cw:
                    nc.scalar.activation(out=v[:, c0 + rw:c0 + cw],
                                         in_=py[:, rw:cw], func=AF.Identity,
                                         bias=c1[:], scale=1.0)
                sq = ck.tile([D, CHUNK], BF16, tag="sq", name=f"sq{l}2{ci}")
                nc.scalar.activation(out=sq[:, :rw], in_=v[:, c0:c0 + rw],
                                     func=AF.Square,
                                     accum_out=ssq2[:, ci:ci + 1])

            a2 = sp.tile([D, 1], F32, tag="co", name=f"a2_{l}")
            c2 = sp.tile([D, 1], F32, tag="co", name=f"c2_{l}")
            finish_stats(ssum2, ssq2, l, 2, a2, c2)

            if l == 0:
                # ---- x'' = a2*v + c2; transpose to node-major; AllGather
                xnew = bufB
                for c0, cw in chunks:
                    sl = slice(c0, c0 + cw)
                    nc.vector.tensor_scalar(out=xnew[:, sl], in0=v[:, sl],
                                            scalar1=a2[:], scalar2=c2[:],
                                            op0=ALU.mult, op1=ALU.add)
                if sh > sh_real:
                    nc.vector.memset(xnew[:, sh_real:sh], 0.0)
                for t in range(nt):
                    ps = psB.tile([128, 128], BF16, tag="tr", name=f"tv{t}")
                    nc.tensor.transpose(ps[:], xnew[:, t * 128:(t + 1) * 128],
                                        ident_b[:])
                    vT = sp.tile([128, D], F16, tag="vT", name=f"vT{t}")
                    nc.scalar.activation(out=vT[:], in_=ps[:], func=AF.Copy)
                    nc.sync.dma_start(out=vshard[t * 128:(t + 1) * 128, :],
                                      in_=vT[:])
                nc.gpsimd.collective_compute(
                    "AllGather", ALU.bypass, replica_groups=rg,
                    ins=[vshard[:]], outs=[vtab[:]])
            else:
                # ---- fold BN2 affine into the classifier
                clswp = pp.tile([D, 16], BF16, tag="clswp", name="clswp")
                nc.vector.tensor_scalar_mul(out=clswp[:], in0=clsw_sb[:],
                                            scalar1=a2[:])
                c2b = pp.tile([D, 1], BF16, tag="cb", name="c2b")
                nc.scalar.activation(out=c2b[:], in_=c2[:], func=AF.Copy)
                pcb = psA.tile([D, CHUNK], F32, tag="mm_gcn", name="pcb")
                nc.tensor.matmul(pcb[:16, 0:1], clsw_sb[:], c2b[:],
                                 start=True, stop=True)
                clsb2 = pp.tile([16, 1], F32, tag="clsb2", name="clsb2")
                nc.scalar.activation(out=clsb2[:], in_=pcb[:16, 0:1],
                                     func=AF.Identity, bias=clsb_sb[:],
                                     scale=1.0)
                out_sb = wp.tile([16, sh], F32, name="out_sb")
                for c0, cw in chunks:
                    sl = slice(c0, c0 + cw)
                    pc = psA.tile([D, CHUNK], F32, tag="mm_gcn",
                                  name=f"pc{c0}")
                    nc.tensor.matmul(pc[:16, :cw], clswp[:], v[:, sl],
                                     start=True, stop=True)
                    nc.scalar.activation(out=out_sb[:, sl], in_=pc[:16, :cw],
                                         func=AF.Identity, bias=clsb2[:],
                                         scale=1.0)
                nc.sync.dma_start(out=out_d[:], in_=out_sb[:])

    nc.compile()
    return nc


# ----------------------------------------------------------------------------
# Entry points
# ----------------------------------------------------------------------------

def _make_in_maps(cfg, inputs):
    W_gcn = np.asarray(inputs["W_gcn"], np.float32)
    b_gcn = np.asarray(inputs["b_gcn"], np.float32)
    ff_w1 = np.asarray(inputs["ff_w1"], np.float32)
    ff_b1 = np.asarray(inputs["ff_b1"], np.float32)
    ff_w2 = np.asarray(inputs["ff_w2"], np.float32)
    cls_w = np.asarray(inputs["cls_w"], np.float32)
    cls_b = np.asarray(inputs["cls_b"], np.float32)

    shared = {
        "clsw": np.ascontiguousarray(cls_w).astype(NP_BF16),
        "clsb": np.ascontiguousarray(cls_b.reshape(16, 1)),
    }
    for l in range(DEPTH):
        shared[f"wg{l}"] = np.ascontiguousarray(W_gcn[l]).astype(NP_BF16)
        shared[f"bgT{l}"] = np.ascontiguousarray(
            b_gcn[l].reshape(1, D)).astype(NP_BF16)
        shared[f"w1_{l}"] = np.ascontiguousarray(ff_w1[l]).astype(NP_BF16)
        shared[f"fb1_{l}"] = np.ascontiguousarray(
            ff_b1[l].reshape(H // D, D).T)
        shared[f"w2_{l}"] = np.ascontiguousarray(ff_w2[l]).astype(NP_BF16)
        shared[f"g1_{l}"] = np.ascontiguousarray(
            np.asarray(inputs["bn1_g"], np.float32)[l].reshape(D, 1))
        shared[f"b1_{l}"] = np.ascontiguousarray(
            np.asarray(inputs["bn1_b"], np.float32)[l].reshape(D, 1))
        shared[f"g2_{l}"] = np.ascontiguousarray(
            np.asarray(inputs["bn2_g"], np.float32)[l].reshape(D, 1))
        shared[f"b2_{l}"] = np.ascontiguousarray(
            np.asarray(inputs["bn2_b"], np.float32)[l].reshape(D, 1))

    sh = cfg["sh"]
    in_maps = []
    for c in range(CORES):
        m = dict(shared)
        m["x0_fm"] = np.ascontiguousarray(
            cfg["table0"][c * sh:(c + 1) * sh].T).astype(NP_BF16)
        m["pay1"] = cfg["pay1"][c]
        m["gidx16"] = np.ascontiguousarray(cfg["gidx16"][c])
        m["invdeg"] = np.ascontiguousarray(cfg["invdeg"][c])
        in_maps.append(m)
    return in_maps


def _postprocess(cfg, results):
    sh, sh_real = cfg["sh"], cfg["sh_real"]
    N = cfg["N"]
    node_of_tok = cfg["node_of_tok"]
    out = np.empty((N, 16), np.float32)
    for c in range(CORES):
        arr = results[c]["out_fm"]  # [16, sh]
        toks = np.arange(c * sh, c * sh + sh_real)
        out[node_of_tok[toks]] = arr.T[:sh_real]
    return out


def _ensure_axon_hooks():
    """The agent image's antenv lacks axon_hooks; synthesize it so
    bass_utils' trace=True path can find the NTFF profile hook."""
    try:
        import antenv.axon_hooks  # noqa: F401
        return
    except ImportError:
        pass
    import types
    import antenv
    mod = types.ModuleType("antenv.axon_hooks")
    mod._hook = None

    def set_axon_ntff_profile_hook(h):
        mod._hook = h

    def get_axon_ntff_profile_hook():
        return mod._hook

    mod.set_axon_ntff_profile_hook = set_axon_ntff_profile_hook
    mod.get_axon_ntff_profile_hook = get_axon_ntff_profile_hook
    sys.modules["antenv.axon_hooks"] = mod
    antenv.axon_hooks = mod
    try:
        from trn_agent_boot.trn_boot import _ntff_profile_via_ctypes
        h = _ntff_profile_via_ctypes("/opt/axon/libaxon_pjrt.so")
        if h is not None:
            mod._hook = h
    except Exception as e:  # pragma: no cover
        print(f"ntff hook setup failed: {e}", file=sys.stderr)


_CACHE = {}


def run(trace=False, **inputs):
    if trace:
        _ensure_axon_hooks()
    nodes = np.asarray(inputs["nodes"], np.float32)
    edge_src = np.asarray(inputs["edge_src"], np.int64)
    edge_dst = np.asarray(inputs["edge_dst"], np.int64)
    cfg = _prepare(nodes, edge_src, edge_dst)

    key = (nodes.shape, len(edge_src), tuple(cfg["K_t"]))
    if key not in _CACHE:
        _CACHE[key] = build_program(cfg)
    nc = _CACHE[key]

    in_maps = _make_in_maps(cfg, inputs)
    res = run_bass_kernel_spmd(nc, in_maps, list(range(CORES)), trace=trace)
    return _postprocess(cfg, res.results), res


def kernel(**inputs) -> np.ndarray:
    out, _ = run(trace=False, **inputs)
    return out


# revision 41
# speedup vs baseline: 1.0762x; 1.0762x over previous
"""Trainium2 Bass kernel for nn_NodeClassifier (gnn_message_passing).

Strategy (8 NeuronCores, SPMD):
  - Nodes block-partitioned by id across 8 cores (6250 each, padded to 6272).
    Within each core's block, nodes are sorted by in-degree so that the
    padded neighbor grid (K-grid) is tight.
  - Edges partitioned by dst core. Per dst 128-node tile, neighbor src
    embeddings arrive FEATURE-MAJOR via one dma_gather(transpose=True) per
    tile from a replicated fp16 node table in DRAM (int16 indices biased by
    -32768 against a base shifted +32768 rows — the gather ucode does signed
    index math, so this addresses all 50176 rows). One 3D tensor_reduce per
    tile then yields the aggregated [feature x node] block directly.
  - Dense per-node compute (GCN linear, BN, FF) runs feature-major in bf16.
    BN affine transforms are folded into the adjacent matmuls (scaled W1 /
    diag residual / scaled cls weights), so only stats cross the BN
    boundary. BN statistics are AllReduced (tiny); the layer-0 output table
    is AllGathered in fp16 for layer 1's gathers. Weights replicated.
"""

import os
import sys
import numpy as np

for _p in ("/opt/trn_rl_repo",):
    if _p not in sys.path and os.path.isdir(_p):
        sys.path.insert(0, _p)

from contextlib import ExitStack

import ml_dtypes

import concourse.bass as bass
import concourse.bacc as bacc
import concourse.mybir as mybir
import concourse.tile as tile
from concourse.bass_utils import run_bass_kernel_spmd
from concourse.masks import make_identity

F32 = mybir.dt.float32
F16 = mybir.dt.float16
BF16 = mybir.dt.bfloat16
I16 = mybir.dt.int16
AF = mybir.ActivationFunctionType
ALU = mybir.AluOpType

NP_BF16 = ml_dtypes.bfloat16

CORES = 8
D = 128
H = 512
DEPTH = 2
EPS = 1e-5
CHUNK = 512  # node-chunk width for the dense phase (one PSUM bank fp32)
IBASE = 32768  # signed-int16 index bias for dma_gather


# ----------------------------------------------------------------------------
# Host-side preparation
# ----------------------------------------------------------------------------

def _prepare(nodes, edge_src, edge_dst):
    """Compute the permutation, sharding and gather schedules from edge data."""
    N = nodes.shape[0]
    assert N % CORES == 0
    sh_real = N // CORES
    nt = -(-sh_real // 128)
    sh = nt * 128
    if sh == sh_real:  # force at least one dummy slot (PAD token row must be 0)
        nt += 1
        sh += 128
    tok_n = CORES * sh
    assert tok_n <= 2 * IBASE, "token space must fit signed-int16 biased range"

    # a dummy (zero-row) token in the upper half of the table: its biased
    # int16 index is positive, so it is safe in the gather's trailing slots
    hi0_core = (IBASE + sh - 1) // sh  # first core whose block is >= IBASE
    pad_tok = hi0_core * sh + sh_real
    assert pad_tok >= IBASE

    deg = np.bincount(edge_dst, minlength=N).astype(np.int64)

    # permutation: per core block, sort nodes by degree ascending
    tok_of_node = np.empty(N, np.int64)
    node_of_tok = np.full(tok_n, -1, np.int64)
    for c in range(CORES):
        ids = np.arange(c * sh_real, (c + 1) * sh_real)
        order = np.argsort(deg[ids], kind="stable")
        toks = c * sh + np.arange(sh_real)
        tok_of_node[ids[order]] = toks
        node_of_tok[toks] = ids[order]

    # group edges by dst token
    dst_tok = tok_of_node[edge_dst]
    src_tok = tok_of_node[edge_src]
    order = np.argsort(dst_tok, kind="stable")
    dst_tok_s = dst_tok[order]
    src_tok_s = src_tok[order]
    cnt_tok = np.bincount(dst_tok_s, minlength=tok_n)
    start_tok = np.concatenate([[0], np.cumsum(cnt_tok)[:-1]])

    # shared K schedule: per tile index t, max over cores of max degree, even
    cnt_mat = cnt_tok.reshape(CORES, nt, 128)
    K_t = cnt_mat.max(axis=(0, 2))
    K_t = np.maximum(K_t, 2)
    K_t = K_t + (K_t % 2)
    koff = np.concatenate([[0], np.cumsum(K_t)])
    ksum = int(koff[-1])

    # per-core gather index grids [128, ksum] (partition = node slot%128)
    gidx = np.full((CORES, 128, ksum), pad_tok, np.int64)
    e_slot = dst_tok_s % sh  # slot within core
    e_core = dst_tok_s // sh
    e_t = e_slot // 128
    e_p = e_slot % 128
    e_r = np.arange(len(dst_tok_s)) - start_tok[dst_tok_s]  # rank within node
    e_col = koff[e_t] + e_r
    gidx[e_core, e_p, e_col] = src_tok_s

    # int16 biased gather index stream for layer 1, as k-major column groups:
    # a dma_gather(transpose=False) with index order i = k*128 + p fills the
    # node-major K-grid [128, cols, D] directly. A single-packet gather hangs
    # at >=1024 indices, so each tile's K columns are split into groups of at
    # most 6 real columns plus one trailing hi-pad column (the gather ucode
    # drops trailing negative indices, and pad rows are zero so the tree
    # reduce can include them). One extra pad column keeps each tile's total
    # column count even for the pairwise tree reduce.
    # Flat index position i lives at partition i%16, column i//16, replicated
    # across the 8 gpsimd q7 cores (partitions 16q+p).
    # The gather ucode drops trailing negative (= lower-half-token) indices,
    # so each instruction's final index must be positive. Where possible we
    # swap an upper-half token into node 127's last slot of the group (sums
    # are order-invariant per node); otherwise the group gets a hi-pad
    # column. Pad rows are zero so the tree reduce just includes them.
    KG = 7  # max columns per gather instruction (128*7 = 896 < 1024 cap)
    parts = []
    groups = []  # per tile: list of (k0, ncols_real, ncols_total, ioff16)
    Kp_t = []  # per tile: total reduced columns (real + pads)
    off = 0
    for t in range(nt):
        K = int(K_t[t])
        kgs = []
        pads = []
        k0 = 0
        while k0 < K:
            kg = min(KG, K - k0)
            # can every core end this group on a positive index (via swap)?
            sl = gidx[:, 127, koff[t] + k0:koff[t] + k0 + kg]
            if (sl >= IBASE).any(axis=1).all():
                for c in range(CORES):
                    j = int(np.argmax(sl[c] >= IBASE))
                    last = kg - 1
                    sl[c, j], sl[c, last] = sl[c, last], sl[c, j]
                pad = 0
            else:
                if kg == KG:
                    kg -= 1
                pad = 1
            kgs.append(kg)
            pads.append(pad)
            k0 += kg
        if (K + sum(pads)) % 2:  # keep the tile's column count even
            for i in range(len(kgs)):
                if kgs[i] + pads[i] < KG:
                    pads[i] += 1
                    break
            else:
                kgs.append(0)
                pads.append(1)
        gl = []
        k0 = 0
        for kg, pad in zip(kgs, pads):
            blk = gidx[:, :, koff[t] + k0:koff[t] + k0 + kg]  # [C, 128, kg]
            blk = blk.transpose(0, 2, 1).reshape(CORES, 128 * kg)
            if pad:
                blk = np.concatenate(
                    [blk, np.full((CORES, 128 * pad), pad_tok, np.int64)],
                    axis=1)
            parts.append(blk)
            gl.append((k0, kg, kg + pad, off))
            off += 128 * (kg + pad) // 16
            k0 += kg
        groups.append(gl)
        Kp_t.append(sum(g[2] for g in gl))
    flat = np.concatenate(parts, axis=1)
    flat16 = (flat - IBASE).astype(np.int16)
    ncol16 = flat.shape[1] // 16
    gidx16 = np.zeros((CORES, 16, ncol16), np.int16)
    pos = np.arange(flat.shape[1])
    gidx16[:, pos % 16, pos // 16] = flat16
    gidx16 = np.tile(gidx16, (1, 8, 1))  # replicate for the 8 gpsimd cores

    # per-core invdeg [128, nt] fp32, node-slot partition layout (0 = dummy)
    deg_tok = cnt_tok.reshape(CORES, sh)
    node_ok = (node_of_tok.reshape(CORES, sh) >= 0)
    iv = (1.0 / np.maximum(deg_tok, 1.0)) * node_ok  # [CORES, sh]
    invdeg = np.zeros((CORES, 128, nt), np.float32)
    for c in range(CORES):
        invdeg[c] = iv[c].reshape(nt, 128).T

    # replicated full node table [tok_n, D], zero at dummy slots
    table0 = np.zeros((tok_n, D), np.float32)
    real = node_of_tok >= 0
    table0[real] = nodes[node_of_tok[real]]

    # host-expanded layer-0 gather payload, node-major [128, ksum*D] fp16
    t16 = table0.astype(np.float16)
    pay1 = t16[gidx]  # [CORES, 128, ksum, D]
    pay1 = np.ascontiguousarray(pay1.reshape(CORES, 128, ksum * D))

    return dict(
        N=N, sh_real=sh_real, sh=sh, nt=nt, tok_n=tok_n,
        K_t=[int(k) for k in K_t], koff=[int(k) for k in koff], ksum=ksum,
        Kp_t=Kp_t, groups=groups, gidx16=gidx16, invdeg=invdeg,
        table0=table0, pay1=pay1, node_of_tok=node_of_tok,
    )


# ----------------------------------------------------------------------------
# Program builder
# ----------------------------------------------------------------------------

def _emit_tree_reduce(nc, G16, G2, K, acc):
    """acc = sum of K [128,D] fp16 chunks of G16, all-fp16 pairwise tree.
    Pass 1 pairs halves of G16 into G2, then in-place halving on G2."""
    ALU_ = mybir.AluOpType
    half = K // 2  # K is even
    if half == 1:
        nc.vector.tensor_tensor(out=acc[:], in0=G16[:, :D],
                                in1=G16[:, D:2 * D], op=ALU_.add)
        return
    nc.vector.tensor_tensor(out=G2[:, :half * D], in0=G16[:, :half * D],
                            in1=G16[:, half * D:K * D], op=ALU_.add)
    width = half
    while width > 2:
        h = width // 2
        if width % 2:
            nc.vector.tensor_tensor(
                out=G2[:, 0:D], in0=G2[:, 0:D],
                in1=G2[:, (width - 1) * D:width * D], op=ALU_.add)
        if h == 1:  # width was 3: after the fold only chunks 0,1 remain
            break
        nc.vector.tensor_tensor(
            out=G2[:, :h * D], in0=G2[:, :h * D],
            in1=G2[:, h * D:2 * h * D], op=ALU_.add)
        width = h
    nc.vector.tensor_tensor(out=acc[:], in0=G2[:, 0:D], in1=G2[:, D:2 * D],
                            op=ALU_.add)


def build_program(cfg):
    nt, sh, sh_real = cfg["nt"], cfg["sh"], cfg["sh_real"]
    tok_n, ksum = cfg["tok_n"], cfg["ksum"]
    K_t, koff, groups = cfg["K_t"], cfg["koff"], cfg["groups"]
    Kp_t = cfg["Kp_t"]
    N = cfg["N"]
    ncol16 = cfg["gidx16"].shape[2]
    kmax = max(max(K_t), max(Kp_t))
    rg = [list(range(CORES))]

    chunks = []
    c0 = 0
    while c0 < sh:
        cw = min(CHUNK, sh - c0)
        chunks.append((c0, cw))
        c0 += cw
    nch = len(chunks)

    nc = bacc.Bacc("TRN2", target_bir_lowering=False, debug=False,
                   num_devices=CORES, num_swdge_queues=4)

    # ---- I/O declarations
    pay_d = nc.dram_tensor("pay1", [128, ksum * D], F16, kind="ExternalInput")
    x0_d = nc.dram_tensor("x0_fm", [D, sh], BF16, kind="ExternalInput")
    gidx_d = nc.dram_tensor("gidx16", [128, ncol16], I16, kind="ExternalInput")
    invdeg_d = nc.dram_tensor("invdeg", [128, nt], F32, kind="ExternalInput")
    wg_d = [nc.dram_tensor(f"wg{l}", [D, D], BF16, kind="ExternalInput")
            for l in range(DEPTH)]
    bgT_d = [nc.dram_tensor(f"bgT{l}", [1, D], BF16, kind="ExternalInput")
             for l in range(DEPTH)]
    w1_d = [nc.dram_tensor(f"w1_{l}", [D, H], BF16, kind="ExternalInput")
            for l in range(DEPTH)]
    fb1_d = [nc.dram_tensor(f"fb1_{l}", [D, H // D], F32, kind="ExternalInput")
             for l in range(DEPTH)]
    w2_d = [nc.dram_tensor(f"w2_{l}", [H, D], BF16, kind="ExternalInput")
            for l in range(DEPTH)]
    bn_d = {}
    for l in range(DEPTH):
        for nm in ("g1", "b1", "g2", "b2"):
            bn_d[(nm, l)] = nc.dram_tensor(f"{nm}_{l}", [D, 1], F32,
                                           kind="ExternalInput")
    clsw_d = nc.dram_tensor("clsw", [D, 16], BF16, kind="ExternalInput")
    clsb_d = nc.dram_tensor("clsb", [16, 1], F32, kind="ExternalInput")
    out_d = nc.dram_tensor("out_fm", [16, sh], F32, kind="ExternalOutput")

    with tile.TileContext(nc) as tc, ExitStack() as ctx:
        dram = ctx.enter_context(tc.tile_pool(name="dram", bufs=1, space="DRAM"))
        wp = ctx.enter_context(tc.tile_pool(name="weights", bufs=1))
        big = ctx.enter_context(tc.tile_pool(name="big", bufs=1))
        gp = ctx.enter_context(tc.tile_pool(name="gather", bufs=8))
        g2p = ctx.enter_context(tc.tile_pool(name="gred", bufs=2))
        pp = ctx.enter_context(tc.tile_pool(name="prep", bufs=2))
        sp = ctx.enter_context(tc.tile_pool(name="small", bufs=4))
        ck = ctx.enter_context(tc.tile_pool(name="chunk", bufs=2))
        psA = ctx.enter_context(tc.tile_pool(name="psA", bufs=1, space="PSUM"))
        psB = ctx.enter_context(tc.tile_pool(name="psB", bufs=2, space="PSUM"))

        # ---- internal DRAM (collective bounce buffers)
        vshard = dram.tile([sh, D], F16, name="vshard")
        vtab = dram.tile([tok_n, D], F16, addr_space="Shared", name="vtab")
        bn_in, bn_out = {}, {}
        for l in range(DEPTH):
            for j in (1, 2):
                bn_in[(l, j)] = dram.tile([D, 2], F32, name=f"bni{l}{j}")
                bn_out[(l, j)] = dram.tile([D, 2], F32, addr_space="Shared",
                                           name=f"bno{l}{j}")

        # ---- load constants / weights to SBUF
        def load(dt_, shape, src, name):
            t = wp.tile(shape, dt_, name=name)
            nc.sync.dma_start(out=t[:], in_=src)
            return t

        gidx_sb = load(I16, [128, ncol16], gidx_d[:], "gidx_sb")
        invdeg_sb = load(F32, [128, nt], invdeg_d[:], "invdeg_sb")
        wg_sb = [load(BF16, [D, D], wg_d[l][:], f"wg_sb{l}") for l in range(DEPTH)]
        bgT_sb = [load(BF16, [1, D], bgT_d[l][:], f"bgT_sb{l}") for l in range(DEPTH)]
        w1_sb = [load(BF16, [D, H], w1_d[l][:], f"w1_sb{l}") for l in range(DEPTH)]
        fb1_sb = [load(F32, [D, H // D], fb1_d[l][:], f"fb1_sb{l}")
                  for l in range(DEPTH)]
        w2_sb = [[load(BF16, [D, D], w2_d[l][h * D:(h + 1) * D, :], f"w2_sb{l}_{h}")
                  for h in range(H // D)] for l in range(DEPTH)]
        bn_sb = {k: load(F32, [D, 1], v[:], f"bn_{k[0]}_{k[1]}")
                 for k, v in bn_d.items()}
        clsw_sb = load(BF16, [D, 16], clsw_d[:], "clsw_sb")
        clsb_sb = load(F32, [16, 1], clsb_d[:], "clsb_sb")

        ident_b = wp.tile([128, 128], BF16, name="ident_b")
        make_identity(nc, ident_b[:])
        ones_row = wp.tile([1, CHUNK], BF16, name="ones_row")
        nc.vector.memset(ones_row[:], 1.0)
        qctr = [0]  # round-robin swdge queue assignment for gathers

        # ---- persistent full-width activations (feature-major [D, sh], bf16)
        bufA = big.tile([D, sh], BF16, name="bufA")  # agg both layers
        bufB = big.tile([D, sh], BF16, name="bufB")  # u(l0) -> xnew(l0)=xres(l1)
        bufC = big.tile([D, sh], BF16, name="bufC")  # x0(l0) -> v(l1)
        bufD = big.tile([D, sh], BF16, name="bufD")  # v(l0) -> u(l1)
        nc.sync.dma_start(out=bufC[:], in_=x0_d[:])

        def bn_vec_math(sums_sb, g_sb, b_sb, a_out, c_out):
            """a = g*rsqrt(var+eps); c = b - mean*a, from [D,2] (sum, sumsq)."""
            m = sp.tile([D, 1], F32, tag="bnv", name="m")
            msq = sp.tile([D, 1], F32, tag="bnv", name="msq")
            var = sp.tile([D, 1], F32, tag="bnv", name="var")
            r = sp.tile([D, 1], F32, tag="bnv", name="r")
            nc.vector.tensor_scalar_mul(out=m[:], in0=sums_sb[:, 0:1],
                                        scalar1=1.0 / N)
            nc.vector.tensor_scalar_mul(out=msq[:], in0=sums_sb[:, 1:2],
                                        scalar1=1.0 / N)
            nc.vector.tensor_tensor(out=var[:], in0=m[:], in1=m[:], op=ALU.mult)
            nc.vector.tensor_tensor(out=var[:], in0=msq[:], in1=var[:],
                                    op=ALU.subtract)
            nc.vector.tensor_scalar_add(out=var[:], in0=var[:], scalar1=EPS)
            nc.vector.reciprocal(out=r[:], in_=var[:])
            nc.scalar.activation(out=a_out[:], in_=r[:], func=AF.Sqrt)
            nc.vector.tensor_tensor(out=a_out[:], in0=g_sb[:], in1=a_out[:],
                                    op=ALU.mult)
            nc.vector.tensor_tensor(out=c_out[:], in0=m[:], in1=a_out[:],
                                    op=ALU.mult)
            nc.vector.tensor_tensor(out=c_out[:], in0=b_sb[:], in1=c_out[:],
                                    op=ALU.subtract)

        def finish_stats(ssum, ssq, l, j, a_out, c_out):
            """Reduce per-chunk partials, AllReduce, compute affine coefs."""
            s2 = sp.tile([D, 2], F32, tag="s2", name=f"s2_{l}{j}")
            nc.vector.tensor_reduce(out=s2[:, 0:1], in_=ssum[:],
                                    axis=mybir.AxisListType.X, op=ALU.add)
            nc.vector.tensor_reduce(out=s2[:, 1:2], in_=ssq[:],
                                    axis=mybir.AxisListType.X, op=ALU.add)
            nc.sync.dma_start(out=bn_in[(l, j)][:], in_=s2[:])
            nc.gpsimd.collective_compute(
                "AllReduce", ALU.add, replica_groups=rg,
                ins=[bn_in[(l, j)][:]], outs=[bn_out[(l, j)][:]])
            sums = sp.tile([D, 2], F32, tag="s2", name=f"sums{l}{j}")
            nc.sync.dma_start(out=sums[:], in_=bn_out[(l, j)][:])
            bn_vec_math(sums, bn_sb[(f"g{j}", l)], bn_sb[(f"b{j}", l)],
                        a_out, c_out)

        for l in range(DEPTH):
            agg = bufA
            if l == 0:
                u, xres, v = bufB, bufC, bufD
            else:
                u, xres, v = bufD, bufB, bufC

            # ---- dense sweep 1 chunk emitter (interleaved with aggregation
            # so the PE/Scalar in-order queues overlap the gather phase)
            ssum1 = sp.tile([D, nch], F32, tag="stat", name=f"ssum{l}1")
            ssq1 = sp.tile([D, nch], F32, tag="stat", name=f"ssq{l}1")

            def emit_sweep1_chunk(ci, l=l, u=u, xres=xres, agg=agg,
                                  ssum1=ssum1, ssq1=ssq1):
                c0, cw = chunks[ci]
                sl = slice(c0, c0 + cw)
                rw = max(0, min(cw, sh_real - c0))
                ph = psA.tile([D, CHUNK], F32, tag="mm_gcn", name=f"ph{l}{c0}")
                nc.tensor.matmul(ph[:, :cw], wg_sb[l][:], agg[:, sl],
                                 start=True, stop=False)
                nc.tensor.matmul(ph[:, :cw], bgT_sb[l][:],
                                 ones_row[:, :cw], start=False, stop=False)
                nc.tensor.matmul(ph[:, :cw], ident_b[:], xres[:, sl],
                                 start=False, stop=True)
                if rw == 0:
                    nc.vector.memset(ssum1[:, ci:ci + 1], 0.0)
                    nc.vector.memset(ssq1[:, ci:ci + 1], 0.0)
                    nc.scalar.activation(out=u[:, sl], in_=ph[:, :cw],
                                         func=AF.Copy)
                    return
                nc.scalar.activation(out=u[:, c0:c0 + rw], in_=ph[:, :rw],
                                     func=AF.Copy,
                                     accum_out=ssum1[:, ci:ci + 1])
                if rw < cw:
                    nc.scalar.activation(out=u[:, c0 + rw:c0 + cw],
                                         in_=ph[:, rw:cw], func=AF.Copy)
                sq = ck.tile([D, CHUNK], BF16, tag="sq", name=f"sq{l}1{ci}")
                nc.scalar.activation(out=sq[:, :rw], in_=u[:, c0:c0 + rw],
                                     func=AF.Square,
                                     accum_out=ssq1[:, ci:ci + 1])

            # ---- aggregation: node-major K-grid gather + fp16 tree reduce
            # + invdeg scale + PE transpose to feature-major
            next_chunk = 0
            for t in range(nt):
                K = K_t[t] if l == 0 else Kp_t[t]
                G16 = gp.tile([128, kmax * D], F16, tag="G16", name=f"G{l}_{t}")
                if l == 0:
                    nc.sync.dma_start(
                        out=G16[:, :K * D],
                        in_=pay_d[:, koff[t] * D:(koff[t] + K) * D])
                else:
                    goff = 0
                    for (k0, kg, ncol, io) in groups[t]:
                        nc.gpsimd.dma_gather(
                            out_ap=G16[:, goff * D:(goff + ncol) * D].rearrange(
                                "p (c d) -> p c d", c=ncol, d=D),
                            in_ap=vtab[IBASE:, :],
                            idxs_ap=gidx_sb[:, io:io + 128 * ncol // 16],
                            num_idxs=128 * ncol,
                            num_idxs_reg=128 * ncol,
                            elem_size=D,
                            transpose=False,
                            queue_num=qctr[0] % 4,
                        )
                        qctr[0] += 1
                        goff += ncol
                G2 = g2p.tile([128, (kmax // 2) * D], F16, tag="G2",
                              name=f"G2_{l}_{t}")
                acc = sp.tile([128, D], F16, tag="acc", name=f"acc{l}_{t}")
                with nc.allow_low_precision(reason="fp16 neighbor sums"):
                    _emit_tree_reduce(nc, G16, G2, K, acc)
                acc2 = sp.tile([128, D], BF16, tag="acc2", name=f"acc2{l}_{t}")
                nc.scalar.activation(out=acc2[:], in_=acc[:], func=AF.Copy,
                                     scale=invdeg_sb[:, t:t + 1])
                ps = psB.tile([128, 128], BF16, tag="tr", name=f"tr{l}_{t}")
                nc.tensor.transpose(ps[:], acc2[:], ident_b[:])
                nc.scalar.activation(out=agg[:, t * 128:(t + 1) * 128],
                                     in_=ps[:], func=AF.Copy)
                while (next_chunk < nch and
                       (chunks[next_chunk][0] + chunks[next_chunk][1] - 1)
                       // 128 <= t):
                    emit_sweep1_chunk(next_chunk)
                    next_chunk += 1

            a1 = sp.tile([D, 1], F32, tag="co", name=f"a1_{l}")
            c1 = sp.tile([D, 1], F32, tag="co", name=f"c1_{l}")
            finish_stats(ssum1, ssq1, l, 1, a1, c1)

            # ---- fold BN1 affine (x' = a1*u + c1) into the FF weights
            w1p = pp.tile([D, H], BF16, tag="w1p", name=f"w1p{l}")
            nc.vector.tensor_scalar_mul(out=w1p[:], in0=w1_sb[l][:],
                                        scalar1=a1[:])
            diag1 = pp.tile([128, 128], BF16, tag="diag", name=f"diag{l}")
            nc.vector.tensor_scalar_mul(out=diag1[:], in0=ident_b[:],
                                        scalar1=a1[:])
            c1b = pp.tile([D, 1], BF16, tag="cb", name=f"c1b{l}")
            nc.scalar.activation(out=c1b[:], in_=c1[:], func=AF.Copy)
            pfb = psA.tile([D, CHUNK], F32, tag="mm_ff1_0", name=f"pfb{l}")
            for h in range(H // D):
                nc.tensor.matmul(pfb[:, h:h + 1],
                                 w1_sb[l][:, h * D:(h + 1) * D], c1b[:],
                                 start=True, stop=True)
            fbp = pp.tile([D, H // D], F32, tag="fbp", name=f"fbp{l}")
            nc.vector.tensor_tensor(out=fbp[:], in0=pfb[:, :H // D],
                                    in1=fb1_sb[l][:], op=ALU.add)

            # ---- dense sweep 2: FF -> v = FF(x') + x' (+stats)
            ssum2 = sp.tile([D, nch], F32, tag="stat", name=f"ssum{l}2")
            ssq2 = sp.tile([D, nch], F32, tag="stat", name=f"ssq{l}2")
            for ci, (c0, cw) in enumerate(chunks):
                sl = slice(c0, c0 + cw)
                rw = max(0, min(cw, sh_real - c0))
                py = psA.tile([D, CHUNK], F32, tag="mm_ff2", name=f"py{l}{c0}")
                for h in range(H // D):
                    pr = psA.tile([D, CHUNK], F32, tag=f"mm_ff1_{h}",
                                  name=f"pr{l}{c0}{h}")
                    nc.tensor.matmul(pr[:, :cw], w1p[:, h * D:(h + 1) * D],
                                     u[:, sl], start=True, stop=True)
                    rh = ck.tile([D, CHUNK], BF16, tag=f"relu{h}",
                                 name=f"rh{l}{c0}{h}")
                    nc.vector.tensor_scalar(out=rh[:, :cw], in0=pr[:, :cw],
                                            scalar1=fbp[:, h:h + 1],
                                            scalar2=0.0, op0=ALU.add,
                                            op1=ALU.max)
                    nc.tensor.matmul(py[:, :cw], w2_sb[l][h][:],
                                     rh[:, :cw], start=(h == 0), stop=False)
                nc.tensor.matmul(py[:, :cw], diag1[:], u[:, sl],
                                 start=False, stop=True)
                if rw == 0:
                    nc.vector.memset(ssum2[:, ci:ci + 1], 0.0)
                    nc.vector.memset(ssq2[:, ci:ci + 1], 0.0)
                    nc.scalar.activation(out=v[:, sl], in_=py[:, :cw],
                                         func=AF.Identity, bias=c1[:],
                                         scale=1.0)
                    continue
                nc.scalar.activation(out=v[:, c0:c0 + rw], in_=py[:, :rw],
                                     func=AF.Identity, bias=c1[:], scale=1.0,
                                     accum_out=ssum2[:, ci:ci + 1])
                if rw < cw:
                    nc.scalar.activation(out=v[:, c0 + rw:c0 + cw],
                                         in_=py[:, rw:cw], func=AF.Identity,
                                         bias=c1[:], scale=1.0)
                sq = ck.tile([D, CHUNK], BF16, tag="sq", name=f"sq{l}2{ci}")
                nc.scalar.activation(out=sq[:, :rw], in_=v[:, c0:c0 + rw],
                                     func=AF.Square,
                                     accum_out=ssq2[:, ci:ci + 1])

            a2 = sp.tile([D, 1], F32, tag="co", name=f"a2_{l}")
            c2 = sp.tile([D, 1], F32, tag="co", name=f"c2_{l}")
            finish_stats(ssum2, ssq2, l, 2, a2, c2)

            if l == 0:
                # ---- x'' = a2*v + c2; transpose to node-major; AllGather.
                # Transposes are emitted right after the chunk that produced
                # their columns so the PE queue overlaps the affine sweep.
                xnew = bufB
                t_done = 0
                for ci, (c0, cw) in enumerate(chunks):
                    sl = slice(c0, c0 + cw)
                    nc.vector.tensor_scalar(out=xnew[:, sl], in0=v[:, sl],
                                            scalar1=a2[:], scalar2=c2[:],
                                            op0=ALU.mult, op1=ALU.add)
                    if ci == nch - 1 and sh > sh_real:
                        nc.vector.memset(xnew[:, sh_real:sh], 0.0)
                    while t_done < nt and (t_done + 1) * 128 <= c0 + cw:
                        t = t_done
                        ps = psB.tile([128, 128], BF16, tag="tr",
                                      name=f"tv{t}")
                        nc.tensor.transpose(
                            ps[:], xnew[:, t * 128:(t + 1) * 128], ident_b[:])
                        vT = sp.tile([128, D], F16, tag="vT", name=f"vT{t}")
                        nc.scalar.activation(out=vT[:], in_=ps[:],
                                             func=AF.Copy)
                        nc.sync.dma_start(
                            out=vshard[t * 128:(t + 1) * 128, :], in_=vT[:])
                        t_done += 1
                nc.gpsimd.collective_compute(
                    "AllGather", ALU.bypass, replica_groups=rg,
                    ins=[vshard[:]], outs=[vtab[:]])
            else:
                # ---- fold BN2 affine into the classifier
                clswp = pp.tile([D, 16], BF16, tag="clswp", name="clswp")
                nc.vector.tensor_scalar_mul(out=clswp[:], in0=clsw_sb[:],
                                            scalar1=a2[:])
                c2b = pp.tile([D, 1], BF16, tag="cb", name="c2b")
                nc.scalar.activation(out=c2b[:], in_=c2[:], func=AF.Copy)
                pcb = psA.tile([D, CHUNK], F32, tag="mm_gcn", name="pcb")
                nc.tensor.matmul(pcb[:16, 0:1], clsw_sb[:], c2b[:],
                                 start=True, stop=True)
                clsb2 = pp.tile([16, 1], F32, tag="clsb2", name="clsb2")
                nc.scalar.activation(out=clsb2[:], in_=pcb[:16, 0:1],
                                     func=AF.Identity, bias=clsb_sb[:],
                                     scale=1.0)
                out_sb = wp.tile([16, sh], F32, name="out_sb")
                for c0, cw in chunks:
                    sl = slice(c0, c0 + cw)
                    pc = psA.tile([D, CHUNK], F32, tag="mm_gcn",
                                  name=f"pc{c0}")
                    nc.tensor.matmul(pc[:16, :cw], clswp[:], v[:, sl],
                                     start=True, stop=True)
                    nc.scalar.activation(out=out_sb[:, sl], in_=pc[:16, :cw],
                                         func=AF.Identity, bias=clsb2[:],
                                         scale=1.0)
                nc.sync.dma_start(out=out_d[:], in_=out_sb[:])

    nc.compile()
    return nc


# ----------------------------------------------------------------------------
# Entry points
# ----------------------------------------------------------------------------

def _make_in_maps(cfg, inputs):
    W_gcn = np.asarray(inputs["W_gcn"], np.float32)
    b_gcn = np.asarray(inputs["b_gcn"], np.float32)
    ff_w1 = np.asarray(inputs["ff_w1"], np.float32)
    ff_b1 = np.asarray(inputs["ff_b1"], np.float32)
    ff_w2 = np.asarray(inputs["ff_w2"], np.float32)
    cls_w = np.asarray(inputs["cls_w"], np.float32)
    cls_b = np.asarray(inputs["cls_b"], np.float32)

    shared = {
        "clsw": np.ascontiguousarray(cls_w).astype(NP_BF16),
        "clsb": np.ascontiguousarray(cls_b.reshape(16, 1)),
    }
    for l in range(DEPTH):
        shared[f"wg{l}"] = np.ascontiguousarray(W_gcn[l]).astype(NP_BF16)
        shared[f"bgT{l}"] = np.ascontiguousarray(
            b_gcn[l].reshape(1, D)).astype(NP_BF16)
        shared[f"w1_{l}"] = np.ascontiguousarray(ff_w1[l]).astype(NP_BF16)
        shared[f"fb1_{l}"] = np.ascontiguousarray(
            ff_b1[l].reshape(H // D, D).T)
        shared[f"w2_{l}"] = np.ascontiguousarray(ff_w2[l]).astype(NP_BF16)
        shared[f"g1_{l}"] = np.ascontiguousarray(
            np.asarray(inputs["bn1_g"], np.float32)[l].reshape(D, 1))
        shared[f"b1_{l}"] = np.ascontiguousarray(
            np.asarray(inputs["bn1_b"], np.float32)[l].reshape(D, 1))
        shared[f"g2_{l}"] = np.ascontiguousarray(
            np.asarray(inputs["bn2_g"], np.float32)[l].reshape(D, 1))
        shared[f"b2_{l}"] = np.ascontiguousarray(
            np.asarray(inputs["bn2_b"], np.float32)[l].reshape(D, 1))

    sh = cfg["sh"]
    in_maps = []
    for c in range(CORES):
        m = dict(shared)
        m["x0_fm"] = np.ascontiguousarray(
            cfg["table0"][c * sh:(c + 1) * sh].T).astype(NP_BF16)
        m["pay1"] = cfg["pay1"][c]
        m["gidx16"] = np.ascontiguousarray(cfg["gidx16"][c])
        m["invdeg"] = np.ascontiguousarray(cfg["invdeg"][c])
        in_maps.append(m)
    return in_maps


def _postprocess(cfg, results):
    sh, sh_real = cfg["sh"], cfg["sh_real"]
    N = cfg["N"]
    node_of_tok = cfg["node_of_tok"]
    out = np.empty((N, 16), np.float32)
    for c in range(CORES):
        arr = results[c]["out_fm"]  # [16, sh]
        toks = np.arange(c * sh, c * sh + sh_real)
        out[node_of_tok[toks]] = arr.T[:sh_real]
    return out


def _ensure_axon_hooks():
    """The agent image's antenv lacks axon_hooks; synthesize it so
    bass_utils' trace=True path can find the NTFF profile hook."""
    try:
        import antenv.axon_hooks  # noqa: F401
        return
    except ImportError:
        pass
    import types
    import antenv
    mod = types.ModuleType("antenv.axon_hooks")
    mod._hook = None

    def set_axon_ntff_profile_hook(h):
        mod._hook = h

    def get_axon_ntff_profile_hook():
        return mod._hook

    mod.set_axon_ntff_profile_hook = set_axon_ntff_profile_hook
    mod.get_axon_ntff_profile_hook = get_axon_ntff_profile_hook
    sys.modules["antenv.axon_hooks"] = mod
    antenv.axon_hooks = mod
    try:
        from trn_agent_boot.trn_boot import _ntff_profile_via_ctypes
        h = _ntff_profile_via_ctypes("/opt/axon/libaxon_pjrt.so")
        if h is not None:
            mod._hook = h
    except Exception as e:  # pragma: no cover
        print(f"ntff hook setup failed: {e}", file=sys.stderr)


_CACHE = {}


def run(trace=False, **inputs):
    if trace:
        _ensure_axon_hooks()
    nodes = np.asarray(inputs["nodes"], np.float32)
    edge_src = np.asarray(inputs["edge_src"], np.int64)
    edge_dst = np.asarray(inputs["edge_dst"], np.int64)
    cfg = _prepare(nodes, edge_src, edge_dst)

    key = (nodes.shape, len(edge_src), tuple(cfg["K_t"]))
    if key not in _CACHE:
        _CACHE[key] = build_program(cfg)
    nc = _CACHE[key]

    in_maps = _make_in_maps(cfg, inputs)
    res = run_bass_kernel_spmd(nc, in_maps, list(range(CORES)), trace=trace)
    return _postprocess(cfg, res.results), res


def kernel(**inputs) -> np.ndarray:
    out, _ = run(trace=False, **inputs)
    return out


# revision 43
# speedup vs baseline: 1.0833x; 1.0066x over previous
"""Trainium2 Bass kernel for nn_NodeClassifier (gnn_message_passing).

Strategy (8 NeuronCores, SPMD):
  - Nodes block-partitioned by id across 8 cores (6250 each, padded to 6272).
    Within each core's block, nodes are sorted by in-degree so that the
    padded neighbor grid (K-grid) is tight.
  - Edges partitioned by dst core. Per dst 128-node tile, neighbor src
    embeddings arrive FEATURE-MAJOR via one dma_gather(transpose=True) per
    tile from a replicated fp16 node table in DRAM (int16 indices biased by
    -32768 against a base shifted +32768 rows — the gather ucode does signed
    index math, so this addresses all 50176 rows). One 3D tensor_reduce per
    tile then yields the aggregated [feature x node] block directly.
  - Dense per-node compute (GCN linear, BN, FF) runs feature-major in bf16.
    BN affine transforms are folded into the adjacent matmuls (scaled W1 /
    diag residual / scaled cls weights), so only stats cross the BN
    boundary. BN statistics are AllReduced (tiny); the layer-0 output table
    is AllGathered in fp16 for layer 1's gathers. Weights replicated.
"""

import os
import sys
import numpy as np

for _p in ("/opt/trn_rl_repo",):
    if _p not in sys.path and os.path.isdir(_p):
        sys.path.insert(0, _p)

from contextlib import ExitStack

import ml_dtypes

import concourse.bass as bass
import concourse.bacc as bacc
import concourse.mybir as mybir
import concourse.tile as tile
from concourse.bass_utils import run_bass_kernel_spmd
from concourse.masks import make_identity

F32 = mybir.dt.float32
F16 = mybir.dt.float16
BF16 = mybir.dt.bfloat16
I16 = mybir.dt.int16
AF = mybir.ActivationFunctionType
ALU = mybir.AluOpType

NP_BF16 = ml_dtypes.bfloat16

CORES = 8
D = 128
H = 512
DEPTH = 2
EPS = 1e-5
CHUNK = 512  # node-chunk width for the dense phase (one PSUM bank fp32)
IBASE = 32768  # signed-int16 index bias for dma_gather


# ----------------------------------------------------------------------------
# Host-side preparation
# ----------------------------------------------------------------------------

def _prepare(nodes, edge_src, edge_dst):
    """Compute the permutation, sharding and gather schedules from edge data."""
    N = nodes.shape[0]
    assert N % CORES == 0
    sh_real = N // CORES
    nt = -(-sh_real // 128)
    sh = nt * 128
    if sh == sh_real:  # force at least one dummy slot (PAD token row must be 0)
        nt += 1
        sh += 128
    tok_n = CORES * sh
    assert tok_n <= 2 * IBASE, "token space must fit signed-int16 biased range"

    # a dummy (zero-row) token in the upper half of the table: its biased
    # int16 index is positive, so it is safe in the gather's trailing slots
    hi0_core = (IBASE + sh - 1) // sh  # first core whose block is >= IBASE
    pad_tok = hi0_core * sh + sh_real
    assert pad_tok >= IBASE

    deg = np.bincount(edge_dst, minlength=N).astype(np.int64)

    # permutation: per core block, sort nodes by degree ascending
    tok_of_node = np.empty(N, np.int64)
    node_of_tok = np.full(tok_n, -1, np.int64)
    for c in range(CORES):
        ids = np.arange(c * sh_real, (c + 1) * sh_real)
        order = np.argsort(deg[ids], kind="stable")
        toks = c * sh + np.arange(sh_real)
        tok_of_node[ids[order]] = toks
        node_of_tok[toks] = ids[order]

    # group edges by dst token
    dst_tok = tok_of_node[edge_dst]
    src_tok = tok_of_node[edge_src]
    order = np.argsort(dst_tok, kind="stable")
    dst_tok_s = dst_tok[order]
    src_tok_s = src_tok[order]
    cnt_tok = np.bincount(dst_tok_s, minlength=tok_n)
    start_tok = np.concatenate([[0], np.cumsum(cnt_tok)[:-1]])

    # shared K schedule: per tile index t, max over cores of max degree, even
    cnt_mat = cnt_tok.reshape(CORES, nt, 128)
    K_t = cnt_mat.max(axis=(0, 2))
    K_t = np.maximum(K_t, 2)
    K_t = K_t + (K_t % 2)
    koff = np.concatenate([[0], np.cumsum(K_t)])
    ksum = int(koff[-1])

    # per-core gather index grids [128, ksum] (partition = node slot%128)
    gidx = np.full((CORES, 128, ksum), pad_tok, np.int64)
    e_slot = dst_tok_s % sh  # slot within core
    e_core = dst_tok_s // sh
    e_t = e_slot // 128
    e_p = e_slot % 128
    e_r = np.arange(len(dst_tok_s)) - start_tok[dst_tok_s]  # rank within node
    e_col = koff[e_t] + e_r
    gidx[e_core, e_p, e_col] = src_tok_s

    # int16 biased gather index stream for layer 1, as k-major column groups:
    # a dma_gather(transpose=False) with index order i = k*128 + p fills the
    # node-major K-grid [128, cols, D] directly. A single-packet gather hangs
    # at >=1024 indices, so each tile's K columns are split into groups of at
    # most 6 real columns plus one trailing hi-pad column (the gather ucode
    # drops trailing negative indices, and pad rows are zero so the tree
    # reduce can include them). One extra pad column keeps each tile's total
    # column count even for the pairwise tree reduce.
    # Flat index position i lives at partition i%16, column i//16, replicated
    # across the 8 gpsimd q7 cores (partitions 16q+p).
    # The gather ucode drops trailing negative (= lower-half-token) indices,
    # so each instruction's final index must be positive. Where possible we
    # swap an upper-half token into node 127's last slot of the group (sums
    # are order-invariant per node); otherwise the group gets a hi-pad
    # column. Pad rows are zero so the tree reduce just includes them.
    KG = 7  # max columns per gather instruction (128*7 = 896 < 1024 cap)
    parts = []
    groups = []  # per tile: list of (k0, ncols_real, ncols_total, ioff16)
    Kp_t = []  # per tile: total reduced columns (real + pads)
    off = 0
    for t in range(nt):
        K = int(K_t[t])
        kgs = []
        pads = []
        k0 = 0
        while k0 < K:
            kg = min(KG, K - k0)
            # can every core end this group on a positive index (via swap)?
            sl = gidx[:, 127, koff[t] + k0:koff[t] + k0 + kg]
            if (sl >= IBASE).any(axis=1).all():
                for c in range(CORES):
                    j = int(np.argmax(sl[c] >= IBASE))
                    last = kg - 1
                    sl[c, j], sl[c, last] = sl[c, last], sl[c, j]
                pad = 0
            else:
                if kg == KG:
                    kg -= 1
                pad = 1
            kgs.append(kg)
            pads.append(pad)
            k0 += kg
        if (K + sum(pads)) % 2:  # keep the tile's column count even
            for i in range(len(kgs)):
                if kgs[i] + pads[i] < KG:
                    pads[i] += 1
                    break
            else:
                kgs.append(0)
                pads.append(1)
        gl = []
        k0 = 0
        for kg, pad in zip(kgs, pads):
            blk = gidx[:, :, koff[t] + k0:koff[t] + k0 + kg]  # [C, 128, kg]
            blk = blk.transpose(0, 2, 1).reshape(CORES, 128 * kg)
            if pad:
                blk = np.concatenate(
                    [blk, np.full((CORES, 128 * pad), pad_tok, np.int64)],
                    axis=1)
            parts.append(blk)
            gl.append((k0, kg, kg + pad, off))
            off += 128 * (kg + pad) // 16
            k0 += kg
        groups.append(gl)
        Kp_t.append(sum(g[2] for g in gl))
    flat = np.concatenate(parts, axis=1)
    flat16 = (flat - IBASE).astype(np.int16)
    ncol16 = flat.shape[1] // 16
    gidx16 = np.zeros((CORES, 16, ncol16), np.int16)
    pos = np.arange(flat.shape[1])
    gidx16[:, pos % 16, pos // 16] = flat16
    gidx16 = np.tile(gidx16, (1, 8, 1))  # replicate for the 8 gpsimd cores

    # per-core invdeg [128, nt] fp32, node-slot partition layout (0 = dummy)
    deg_tok = cnt_tok.reshape(CORES, sh)
    node_ok = (node_of_tok.reshape(CORES, sh) >= 0)
    iv = (1.0 / np.maximum(deg_tok, 1.0)) * node_ok  # [CORES, sh]
    invdeg = np.zeros((CORES, 128, nt), np.float32)
    for c in range(CORES):
        invdeg[c] = iv[c].reshape(nt, 128).T

    # replicated full node table [tok_n, D], zero at dummy slots
    table0 = np.zeros((tok_n, D), np.float32)
    real = node_of_tok >= 0
    table0[real] = nodes[node_of_tok[real]]

    # host-expanded layer-0 gather payload, node-major [128, ksum*D] fp16
    t16 = table0.astype(np.float16)
    pay1 = t16[gidx]  # [CORES, 128, ksum, D]
    pay1 = np.ascontiguousarray(pay1.reshape(CORES, 128, ksum * D))

    return dict(
        N=N, sh_real=sh_real, sh=sh, nt=nt, tok_n=tok_n,
        K_t=[int(k) for k in K_t], koff=[int(k) for k in koff], ksum=ksum,
        Kp_t=Kp_t, groups=groups, gidx16=gidx16, invdeg=invdeg,
        table0=table0, pay1=pay1, node_of_tok=node_of_tok,
    )


# ----------------------------------------------------------------------------
# Program builder
# ----------------------------------------------------------------------------

def _emit_tree_reduce(nc, G16, G2, K, acc):
    """acc = sum of K [128,D] fp16 chunks of G16, all-fp16 pairwise tree.
    Pass 1 pairs halves of G16 into G2, then in-place halving on G2."""
    ALU_ = mybir.AluOpType
    half = K // 2  # K is even
    if half == 1:
        nc.vector.tensor_tensor(out=acc[:], in0=G16[:, :D],
                                in1=G16[:, D:2 * D], op=ALU_.add)
        return
    nc.vector.tensor_tensor(out=G2[:, :half * D], in0=G16[:, :half * D],
                            in1=G16[:, half * D:K * D], op=ALU_.add)
    width = half
    while width > 2:
        h = width // 2
        if width % 2:
            nc.vector.tensor_tensor(
                out=G2[:, 0:D], in0=G2[:, 0:D],
                in1=G2[:, (width - 1) * D:width * D], op=ALU_.add)
        if h == 1:  # width was 3: after the fold only chunks 0,1 remain
            break
        nc.vector.tensor_tensor(
            out=G2[:, :h * D], in0=G2[:, :h * D],
            in1=G2[:, h * D:2 * h * D], op=ALU_.add)
        width = h
    nc.vector.tensor_tensor(out=acc[:], in0=G2[:, 0:D], in1=G2[:, D:2 * D],
                            op=ALU_.add)


def build_program(cfg):
    nt, sh, sh_real = cfg["nt"], cfg["sh"], cfg["sh_real"]
    tok_n, ksum = cfg["tok_n"], cfg["ksum"]
    K_t, koff, groups = cfg["K_t"], cfg["koff"], cfg["groups"]
    Kp_t = cfg["Kp_t"]
    N = cfg["N"]
    ncol16 = cfg["gidx16"].shape[2]
    kmax = max(max(K_t), max(Kp_t))
    rg = [list(range(CORES))]

    chunks = []
    c0 = 0
    while c0 < sh:
        cw = min(CHUNK, sh - c0)
        chunks.append((c0, cw))
        c0 += cw
    nch = len(chunks)

    nc = bacc.Bacc("TRN2", target_bir_lowering=False, debug=False,
                   num_devices=CORES, num_swdge_queues=4)

    # ---- I/O declarations
    pay_d = nc.dram_tensor("pay1", [128, ksum * D], F16, kind="ExternalInput")
    x0_d = nc.dram_tensor("x0_fm", [D, sh], BF16, kind="ExternalInput")
    gidx_d = nc.dram_tensor("gidx16", [128, ncol16], I16, kind="ExternalInput")
    invdeg_d = nc.dram_tensor("invdeg", [128, nt], F32, kind="ExternalInput")
    wg_d = [nc.dram_tensor(f"wg{l}", [D, D], BF16, kind="ExternalInput")
            for l in range(DEPTH)]
    bgT_d = [nc.dram_tensor(f"bgT{l}", [1, D], BF16, kind="ExternalInput")
             for l in range(DEPTH)]
    w1_d = [nc.dram_tensor(f"w1_{l}", [D, H], BF16, kind="ExternalInput")
            for l in range(DEPTH)]
    fb1_d = [nc.dram_tensor(f"fb1_{l}", [D, H // D], F32, kind="ExternalInput")
             for l in range(DEPTH)]
    w2_d = [nc.dram_tensor(f"w2_{l}", [H, D], BF16, kind="ExternalInput")
            for l in range(DEPTH)]
    bn_d = {}
    for l in range(DEPTH):
        for nm in ("g1", "b1", "g2", "b2"):
            bn_d[(nm, l)] = nc.dram_tensor(f"{nm}_{l}", [D, 1], F32,
                                           kind="ExternalInput")
    clsw_d = nc.dram_tensor("clsw", [D, 16], BF16, kind="ExternalInput")
    clsb_d = nc.dram_tensor("clsb", [16, 1], F32, kind="ExternalInput")
    out_d = nc.dram_tensor("out_fm", [16, sh], F32, kind="ExternalOutput")

    with tile.TileContext(nc) as tc, ExitStack() as ctx:
        dram = ctx.enter_context(tc.tile_pool(name="dram", bufs=1, space="DRAM"))
        wp = ctx.enter_context(tc.tile_pool(name="weights", bufs=1))
        big = ctx.enter_context(tc.tile_pool(name="big", bufs=1))
        gp = ctx.enter_context(tc.tile_pool(name="gather", bufs=8))
        g2p = ctx.enter_context(tc.tile_pool(name="gred", bufs=2))
        pp = ctx.enter_context(tc.tile_pool(name="prep", bufs=2))
        sp = ctx.enter_context(tc.tile_pool(name="small", bufs=4))
        ck = ctx.enter_context(tc.tile_pool(name="chunk", bufs=2))
        psA = ctx.enter_context(tc.tile_pool(name="psA", bufs=1, space="PSUM"))
        psB = ctx.enter_context(tc.tile_pool(name="psB", bufs=2, space="PSUM"))

        # ---- internal DRAM (collective bounce buffers)
        vshard = dram.tile([sh, D], F16, name="vshard")
        vtab = dram.tile([tok_n, D], F16, addr_space="Shared", name="vtab")
        bn_in, bn_out = {}, {}
        for l in range(DEPTH):
            for j in (1, 2):
                bn_in[(l, j)] = dram.tile([D, 2], F32, name=f"bni{l}{j}")
                bn_out[(l, j)] = dram.tile([D, 2], F32, addr_space="Shared",
                                           name=f"bno{l}{j}")

        # ---- load constants / weights to SBUF
        def load(dt_, shape, src, name):
            t = wp.tile(shape, dt_, name=name)
            nc.sync.dma_start(out=t[:], in_=src)
            return t

        gidx_sb = load(I16, [128, ncol16], gidx_d[:], "gidx_sb")
        invdeg_sb = load(F32, [128, nt], invdeg_d[:], "invdeg_sb")
        wg_sb = [load(BF16, [D, D], wg_d[l][:], f"wg_sb{l}") for l in range(DEPTH)]
        bgT_sb = [load(BF16, [1, D], bgT_d[l][:], f"bgT_sb{l}") for l in range(DEPTH)]
        w1_sb = [load(BF16, [D, H], w1_d[l][:], f"w1_sb{l}") for l in range(DEPTH)]
        fb1_sb = [load(F32, [D, H // D], fb1_d[l][:], f"fb1_sb{l}")
                  for l in range(DEPTH)]
        w2_sb = [[load(BF16, [D, D], w2_d[l][h * D:(h + 1) * D, :], f"w2_sb{l}_{h}")
                  for h in range(H // D)] for l in range(DEPTH)]
        bn_sb = {k: load(F32, [D, 1], v[:], f"bn_{k[0]}_{k[1]}")
                 for k, v in bn_d.items()}
        clsw_sb = load(BF16, [D, 16], clsw_d[:], "clsw_sb")
        clsb_sb = load(F32, [16, 1], clsb_d[:], "clsb_sb")

        ident_b = wp.tile([128, 128], BF16, name="ident_b")
        make_identity(nc, ident_b[:])
        ones_row = wp.tile([1, CHUNK], BF16, name="ones_row")
        nc.vector.memset(ones_row[:], 1.0)
        qctr = [0]  # round-robin swdge queue assignment for gathers

        # ---- persistent full-width activations (feature-major [D, sh], bf16)
        bufA = big.tile([D, sh], BF16, name="bufA")  # agg both layers
        bufB = big.tile([D, sh], BF16, name="bufB")  # u(l0) -> xnew(l0)=xres(l1)
        bufC = big.tile([D, sh], BF16, name="bufC")  # x0(l0) -> v(l1)
        bufD = big.tile([D, sh], BF16, name="bufD")  # v(l0) -> u(l1)
        nc.sync.dma_start(out=bufC[:], in_=x0_d[:])

        def bn_vec_math(sums_sb, g_sb, b_sb, a_out, c_out):
            """a = g*rsqrt(var+eps); c = b - mean*a, from [D,2] (sum, sumsq)."""
            m = sp.tile([D, 1], F32, tag="bnv", name="m")
            msq = sp.tile([D, 1], F32, tag="bnv", name="msq")
            var = sp.tile([D, 1], F32, tag="bnv", name="var")
            r = sp.tile([D, 1], F32, tag="bnv", name="r")
            nc.vector.tensor_scalar_mul(out=m[:], in0=sums_sb[:, 0:1],
                                        scalar1=1.0 / N)
            nc.vector.tensor_scalar_mul(out=msq[:], in0=sums_sb[:, 1:2],
                                        scalar1=1.0 / N)
            nc.vector.tensor_tensor(out=var[:], in0=m[:], in1=m[:], op=ALU.mult)
            nc.vector.tensor_tensor(out=var[:], in0=msq[:], in1=var[:],
                                    op=ALU.subtract)
            nc.vector.tensor_scalar_add(out=var[:], in0=var[:], scalar1=EPS)
            nc.vector.reciprocal(out=r[:], in_=var[:])
            nc.scalar.activation(out=a_out[:], in_=r[:], func=AF.Sqrt)
            nc.vector.tensor_tensor(out=a_out[:], in0=g_sb[:], in1=a_out[:],
                                    op=ALU.mult)
            nc.vector.tensor_tensor(out=c_out[:], in0=m[:], in1=a_out[:],
                                    op=ALU.mult)
            nc.vector.tensor_tensor(out=c_out[:], in0=b_sb[:], in1=c_out[:],
                                    op=ALU.subtract)

        def finish_stats(ssum, ssq, l, j, a_out, c_out):
            """Reduce per-chunk partials, AllReduce, compute affine coefs."""
            s2 = sp.tile([D, 2], F32, tag="s2", name=f"s2_{l}{j}")
            nc.vector.tensor_reduce(out=s2[:, 0:1], in_=ssum[:],
                                    axis=mybir.AxisListType.X, op=ALU.add)
            nc.vector.tensor_reduce(out=s2[:, 1:2], in_=ssq[:],
                                    axis=mybir.AxisListType.X, op=ALU.add)
            nc.sync.dma_start(out=bn_in[(l, j)][:], in_=s2[:])
            nc.gpsimd.collective_compute(
                "AllReduce", ALU.add, replica_groups=rg,
                ins=[bn_in[(l, j)][:]], outs=[bn_out[(l, j)][:]])
            sums = sp.tile([D, 2], F32, tag="s2", name=f"sums{l}{j}")
            nc.sync.dma_start(out=sums[:], in_=bn_out[(l, j)][:])
            bn_vec_math(sums, bn_sb[(f"g{j}", l)], bn_sb[(f"b{j}", l)],
                        a_out, c_out)

        for l in range(DEPTH):
            agg = bufA
            if l == 0:
                u, xres, v = bufB, bufC, bufD
            else:
                u, xres, v = bufD, bufB, bufC

            # ---- dense sweep 1 chunk emitter (interleaved with aggregation
            # so the PE/Scalar in-order queues overlap the gather phase)
            ssum1 = sp.tile([D, nch], F32, tag="stat", name=f"ssum{l}1")
            ssq1 = sp.tile([D, nch], F32, tag="stat", name=f"ssq{l}1")

            def emit_sweep1_chunk(ci, l=l, u=u, xres=xres, agg=agg,
                                  ssum1=ssum1, ssq1=ssq1):
                c0, cw = chunks[ci]
                sl = slice(c0, c0 + cw)
                rw = max(0, min(cw, sh_real - c0))
                ph = psA.tile([D, CHUNK], F32, tag="mm_gcn", name=f"ph{l}{c0}")
                nc.tensor.matmul(ph[:, :cw], wg_sb[l][:], agg[:, sl],
                                 start=True, stop=False)
                nc.tensor.matmul(ph[:, :cw], bgT_sb[l][:],
                                 ones_row[:, :cw], start=False, stop=False)
                nc.tensor.matmul(ph[:, :cw], ident_b[:], xres[:, sl],
                                 start=False, stop=True)
                if rw == 0:
                    nc.vector.memset(ssum1[:, ci:ci + 1], 0.0)
                    nc.vector.memset(ssq1[:, ci:ci + 1], 0.0)
                    nc.scalar.activation(out=u[:, sl], in_=ph[:, :cw],
                                         func=AF.Copy)
                    return
                nc.scalar.activation(out=u[:, c0:c0 + rw], in_=ph[:, :rw],
                                     func=AF.Copy,
                                     accum_out=ssum1[:, ci:ci + 1])
                if rw < cw:
                    nc.scalar.activation(out=u[:, c0 + rw:c0 + cw],
                                         in_=ph[:, rw:cw], func=AF.Copy)
                sq = ck.tile([D, CHUNK], BF16, tag="sq", name=f"sq{l}1{ci}")
                nc.scalar.activation(out=sq[:, :rw], in_=u[:, c0:c0 + rw],
                                     func=AF.Square,
                                     accum_out=ssq1[:, ci:ci + 1])

            # ---- aggregation: node-major K-grid gather + fp16 tree reduce
            # + invdeg scale + PE transpose to feature-major
            next_chunk = 0
            for t in range(nt):
                K = K_t[t] if l == 0 else Kp_t[t]
                G16 = gp.tile([128, kmax * D], F16, tag="G16", name=f"G{l}_{t}")
                if l == 0:
                    nc.sync.dma_start(
                        out=G16[:, :K * D],
                        in_=pay_d[:, koff[t] * D:(koff[t] + K) * D])
                else:
                    goff = 0
                    for (k0, kg, ncol, io) in groups[t]:
                        nc.gpsimd.dma_gather(
                            out_ap=G16[:, goff * D:(goff + ncol) * D].rearrange(
                                "p (c d) -> p c d", c=ncol, d=D),
                            in_ap=vtab[IBASE:, :],
                            idxs_ap=gidx_sb[:, io:io + 128 * ncol // 16],
                            num_idxs=128 * ncol,
                            num_idxs_reg=128 * ncol,
                            elem_size=D,
                            transpose=False,
                            queue_num=qctr[0] % 4,
                        )
                        qctr[0] += 1
                        goff += ncol
                G2 = g2p.tile([128, (kmax // 2) * D], F16, tag="G2",
                              name=f"G2_{l}_{t}")
                acc = sp.tile([128, D], F16, tag="acc", name=f"acc{l}_{t}")
                with nc.allow_low_precision(reason="fp16 neighbor sums"):
                    _emit_tree_reduce(nc, G16, G2, K, acc)
                acc2 = sp.tile([128, D], BF16, tag="acc2", name=f"acc2{l}_{t}")
                nc.scalar.activation(out=acc2[:], in_=acc[:], func=AF.Copy,
                                     scale=invdeg_sb[:, t:t + 1])
                ps = psB.tile([128, 128], BF16, tag="tr", name=f"tr{l}_{t}")
                nc.tensor.transpose(ps[:], acc2[:], ident_b[:])
                nc.scalar.activation(out=agg[:, t * 128:(t + 1) * 128],
                                     in_=ps[:], func=AF.Copy)
                while (next_chunk < nch and
                       (chunks[next_chunk][0] + chunks[next_chunk][1] - 1)
                       // 128 <= t):
                    emit_sweep1_chunk(next_chunk)
                    next_chunk += 1

            a1 = sp.tile([D, 1], F32, tag="co", name=f"a1_{l}")
            c1 = sp.tile([D, 1], F32, tag="co", name=f"c1_{l}")
            finish_stats(ssum1, ssq1, l, 1, a1, c1)

            # ---- fold BN1 affine (x' = a1*u + c1) into the FF weights
            w1p = pp.tile([D, H], BF16, tag="w1p", name=f"w1p{l}")
            nc.vector.tensor_scalar_mul(out=w1p[:], in0=w1_sb[l][:],
                                        scalar1=a1[:])
            diag1 = pp.tile([128, 128], BF16, tag="diag", name=f"diag{l}")
            nc.vector.tensor_scalar_mul(out=diag1[:], in0=ident_b[:],
                                        scalar1=a1[:])
            c1b = pp.tile([D, 1], BF16, tag="cb", name=f"c1b{l}")
            nc.scalar.activation(out=c1b[:], in_=c1[:], func=AF.Copy)
            pfb = psA.tile([D, CHUNK], F32, tag="mm_ff1_0", name=f"pfb{l}")
            for h in range(H // D):
                nc.tensor.matmul(pfb[:, h:h + 1],
                                 w1_sb[l][:, h * D:(h + 1) * D], c1b[:],
                                 start=True, stop=True)
            fbp = pp.tile([D, H // D], F32, tag="fbp", name=f"fbp{l}")
            nc.vector.tensor_tensor(out=fbp[:], in0=pfb[:, :H // D],
                                    in1=fb1_sb[l][:], op=ALU.add)

            # ---- dense sweep 2: FF -> v = FF(x') + x' (+stats)
            ssum2 = sp.tile([D, nch], F32, tag="stat", name=f"ssum{l}2")
            ssq2 = sp.tile([D, nch], F32, tag="stat", name=f"ssq{l}2")
            for ci, (c0, cw) in enumerate(chunks):
                sl = slice(c0, c0 + cw)
                rw = max(0, min(cw, sh_real - c0))
                py = psA.tile([D, CHUNK], F32, tag="mm_ff2", name=f"py{l}{c0}")
                for h in range(H // D):
                    pr = psA.tile([D, CHUNK], F32, tag=f"mm_ff1_{h}",
                                  name=f"pr{l}{c0}{h}")
                    nc.tensor.matmul(pr[:, :cw], w1p[:, h * D:(h + 1) * D],
                                     u[:, sl], start=True, stop=True)
                    rh = ck.tile([D, CHUNK], BF16, tag=f"relu{h}",
                                 name=f"rh{l}{c0}{h}")
                    nc.vector.tensor_scalar(out=rh[:, :cw], in0=pr[:, :cw],
                                            scalar1=fbp[:, h:h + 1],
                                            scalar2=0.0, op0=ALU.add,
                                            op1=ALU.max)
                    nc.tensor.matmul(py[:, :cw], w2_sb[l][h][:],
                                     rh[:, :cw], start=(h == 0), stop=False)
                nc.tensor.matmul(py[:, :cw], diag1[:], u[:, sl],
                                 start=False, stop=True)
                if rw == 0:
                    nc.vector.memset(ssum2[:, ci:ci + 1], 0.0)
                    nc.vector.memset(ssq2[:, ci:ci + 1], 0.0)
                    nc.scalar.activation(out=v[:, sl], in_=py[:, :cw],
                                         func=AF.Identity, bias=c1[:],
                                         scale=1.0)
                    continue
                nc.scalar.activation(out=v[:, c0:c0 + rw], in_=py[:, :rw],
                                     func=AF.Identity, bias=c1[:], scale=1.0,
                                     accum_out=ssum2[:, ci:ci + 1])
                if rw < cw:
                    nc.scalar.activation(out=v[:, c0 + rw:c0 + cw],
                                         in_=py[:, rw:cw], func=AF.Identity,
                                         bias=c1[:], scale=1.0)
                sq = ck.tile([D, CHUNK], BF16, tag="sq", name=f"sq{l}2{ci}")
                nc.scalar.activation(out=sq[:, :rw], in_=v[:, c0:c0 + rw],
                                     func=AF.Square,
                                     accum_out=ssq2[:, ci:ci + 1])

            a2 = sp.tile([D, 1], F32, tag="co", name=f"a2_{l}")
            c2 = sp.tile([D, 1], F32, tag="co", name=f"c2_{l}")
            finish_stats(ssum2, ssq2, l, 2, a2, c2)

            if l == 0:
                # ---- x'' = a2*v + c2; transpose to node-major; AllGather.
                # Transposes are emitted right after the chunk that produced
                # their columns so the PE queue overlaps the affine sweep.
                xnew = bufB
                t_done = 0
                for ci, (c0, cw) in enumerate(chunks):
                    sl = slice(c0, c0 + cw)
                    nc.vector.tensor_scalar(out=xnew[:, sl], in0=v[:, sl],
                                            scalar1=a2[:], scalar2=c2[:],
                                            op0=ALU.mult, op1=ALU.add)
                    if ci == nch - 1 and sh > sh_real:
                        nc.vector.memset(xnew[:, sh_real:sh], 0.0)
                    while t_done < nt and (t_done + 1) * 128 <= c0 + cw:
                        t = t_done
                        ps = psB.tile([128, 128], BF16, tag="tr",
                                      name=f"tv{t}")
                        nc.tensor.transpose(
                            ps[:], xnew[:, t * 128:(t + 1) * 128], ident_b[:])
                        vT = sp.tile([128, D], F16, tag="vT", name=f"vT{t}")
                        nc.scalar.activation(out=vT[:], in_=ps[:],
                                             func=AF.Copy)
                        nc.sync.dma_start(
                            out=vshard[t * 128:(t + 1) * 128, :], in_=vT[:])
                        t_done += 1
                nc.gpsimd.collective_compute(
                    "AllGather", ALU.bypass, replica_groups=rg,
                    ins=[vshard[:]], outs=[vtab[:]])
            else:
                # ---- fold BN2 affine into the classifier
                clswp = pp.tile([D, 16], BF16, tag="clswp", name="clswp")
                nc.vector.tensor_scalar_mul(out=clswp[:], in0=clsw_sb[:],
                                            scalar1=a2[:])
                c2b = pp.tile([D, 1], BF16, tag="cb", name="c2b")
                nc.scalar.activation(out=c2b[:], in_=c2[:], func=AF.Copy)
                pcb = psA.tile([D, CHUNK], F32, tag="mm_gcn", name="pcb")
                nc.tensor.matmul(pcb[:16, 0:1], clsw_sb[:], c2b[:],
                                 start=True, stop=True)
                clsb2 = pp.tile([16, 1], F32, tag="clsb2", name="clsb2")
                nc.scalar.activation(out=clsb2[:], in_=pcb[:16, 0:1],
                                     func=AF.Identity, bias=clsb_sb[:],
                                     scale=1.0)
                out_sb = wp.tile([16, sh], F32, name="out_sb")
                for c0, cw in chunks:
                    sl = slice(c0, c0 + cw)
                    pc = psA.tile([D, CHUNK], F32, tag="mm_gcn",
                                  name=f"pc{c0}")
                    nc.tensor.matmul(pc[:16, :cw], clswp[:], v[:, sl],
                                     start=True, stop=True)
                    nc.scalar.activation(out=out_sb[:, sl], in_=pc[:16, :cw],
                                         func=AF.Identity, bias=clsb2[:],
                                         scale=1.0)
                nc.sync.dma_start(out=out_d[:], in_=out_sb[:])

    nc.compile()
    return nc


# ----------------------------------------------------------------------------
# Entry points
# ----------------------------------------------------------------------------

def _make_in_maps(cfg, inputs):
    W_gcn = np.asarray(inputs["W_gcn"], np.float32)
    b_gcn = np.asarray(inputs["b_gcn"], np.float32)
    ff_w1 = np.asarray(inputs["ff_w1"], np.float32)
    ff_b1 = np.asarray(inputs["ff_b1"], np.float32)
    ff_w2 = np.asarray(inputs["ff_w2"], np.float32)
    cls_w = np.asarray(inputs["cls_w"], np.float32)
    cls_b = np.asarray(inputs["cls_b"], np.float32)

    shared = {
        "clsw": np.ascontiguousarray(cls_w).astype(NP_BF16),
        "clsb": np.ascontiguousarray(cls_b.reshape(16, 1)),
    }
    for l in range(DEPTH):
        shared[f"wg{l}"] = np.ascontiguousarray(W_gcn[l]).astype(NP_BF16)
        shared[f"bgT{l}"] = np.ascontiguousarray(
            b_gcn[l].reshape(1, D)).astype(NP_BF16)
        shared[f"w1_{l}"] = np.ascontiguousarray(ff_w1[l]).astype(NP_BF16)
        shared[f"fb1_{l}"] = np.ascontiguousarray(
            ff_b1[l].reshape(H // D, D).T)
        shared[f"w2_{l}"] = np.ascontiguousarray(ff_w2[l]).astype(NP_BF16)
        shared[f"g1_{l}"] = np.ascontiguousarray(
            np.asarray(inputs["bn1_g"], np.float32)[l].reshape(D, 1))
        shared[f"b1_{l}"] = np.ascontiguousarray(
            np.asarray(inputs["bn1_b"], np.float32)[l].reshape(D, 1))
        shared[f"g2_{l}"] = np.ascontiguousarray(
            np.asarray(inputs["bn2_g"], np.float32)[l].reshape(D, 1))
        shared[f"b2_{l}"] = np.ascontiguousarray(
            np.asarray(inputs["bn2_b"], np.float32)[l].reshape(D, 1))

    sh = cfg["sh"]
    in_maps = []
    for c in range(CORES):
        m = dict(shared)
        m["x0_fm"] = np.ascontiguousarray(
            cfg["table0"][c * sh:(c + 1) * sh].T).astype(NP_BF16)
        m["pay1"] = cfg["pay1"][c]
        m["gidx16"] = np.ascontiguousarray(cfg["gidx16"][c])
        m["invdeg"] = np.ascontiguousarray(cfg["invdeg"][c])
        in_maps.append(m)
    return in_maps


def _postprocess(cfg, results):
    sh, sh_real = cfg["sh"], cfg["sh_real"]
    N = cfg["N"]
    node_of_tok = cfg["node_of_tok"]
    out = np.empty((N, 16), np.float32)
    for c in range(CORES):
        arr = results[c]["out_fm"]  # [16, sh]
        toks = np.arange(c * sh, c * sh + sh_real)
        out[node_of_tok[toks]] = arr.T[:sh_real]
    return out


def _ensure_axon_hooks():
    """The agent image's antenv lacks axon_hooks; synthesize it so
    bass_utils' trace=True path can find the NTFF profile hook."""
    try:
        import antenv.axon_hooks  # noqa: F401
        return
    except ImportError:
        pass
    import types
    import antenv
    mod = types.ModuleType("antenv.axon_hooks")
    mod._hook = None

    def set_axon_ntff_profile_hook(h):
        mod._hook = h

    def get_axon_ntff_profile_hook():
        return mod._hook

    mod.set_axon_ntff_profile_hook = set_axon_ntff_profile_hook
    mod.get_axon_ntff_profile_hook = get_axon_ntff_profile_hook
    sys.modules["antenv.axon_hooks"] = mod
    antenv.axon_hooks = mod
    try:
        from trn_agent_boot.trn_boot import _ntff_profile_via_ctypes
        h = _ntff_profile_via_ctypes("/opt/axon/libaxon_pjrt.so")
        if h is not None:
            mod._hook = h
    except Exception as e:  # pragma: no cover
        print(f"ntff hook setup failed: {e}", file=sys.stderr)


_CACHE = {}


def run(trace=False, **inputs):
    if trace:
        _ensure_axon_hooks()
    nodes = np.asarray(inputs["nodes"], np.float32)
    edge_src = np.asarray(inputs["edge_src"], np.int64)
    edge_dst = np.asarray(inputs["edge_dst"], np.int64)
    cfg = _prepare(nodes, edge_src, edge_dst)

    key = (nodes.shape, len(edge_src), tuple(cfg["K_t"]))
    if key not in _CACHE:
        _CACHE[key] = build_program(cfg)
    nc = _CACHE[key]

    in_maps = _make_in_maps(cfg, inputs)
    res = run_bass_kernel_spmd(nc, in_maps, list(range(CORES)), trace=trace)
    return _postprocess(cfg, res.results), res


def kernel(**inputs) -> np.ndarray:
    out, _ = run(trace=False, **inputs)
    return out
